# revision 34
# baseline (speedup 1.0000x reference)
"""Trainium2 Bass kernel for an attention block (dense transformer).

Reference computation (per batch b):
    q = x @ Wq.T + bq ; k = x @ Wk.T + bk ; v = x @ Wv.T + bv
    per head: attn = softmax(q k^T / sqrt(dh)) ; o = attn @ v
    out = concat(o) @ Wo.T + bo + x

Sharding: 8 cores = 4 batches x 2 query-halves (data parallel; K/V
projections duplicated within a pair, which avoids all collectives).

Device-side layouts are feature-major ("transposed"): the host passes
x[b].T and W.T so no on-device fp32 transposes are ever needed.
Matmuls run in float32r (fp32 rounded to E8M11, full-rate on the PE —
4x faster than plain fp32). Matmul operands coming from DRAM are
pre-rounded to fp32r on the host; operands produced on-device are
written with float32r output dtype by ACT/DVE so the BIR verifier's
"rounded producer" rule is satisfied.

Softmax: scoresT[j, m] = K.T-chunk.T @ Q.T, exp on ScalarE (no
max-subtraction needed: |scores| < ~3 for this distribution), and a
ones-column appended to V so P @ [V | 1] yields both P@V and the row
sums in one PSUM accumulation group. bv is folded in after
normalization (attn rows sum to 1).
"""

import os
import sys
from contextlib import ExitStack

import numpy as np

sys.path.insert(0, "/opt/trn_rl_repo")
os.environ.setdefault("MYCRO_LOCAL_CACHE", "1")

import concourse.bass as bass  # noqa: E402
import concourse.tile as tile  # noqa: E402
from concourse import mybir  # noqa: E402
from concourse.bass_utils import run_bass_kernel_spmd  # noqa: E402

# ---------------------------------------------------------------------------
# walrus codegen in this toolchain encodes at most ONE semaphore wait per
# instruction ("Too many sync wait commands").  Tile's scheduler freely emits
# several.  Split every multi-wait sync_info into standalone EventSemaphore
# wait instructions on the same engine, immediately before the instruction —
# semantically identical (engine sequencers execute them in program order).
# ---------------------------------------------------------------------------
import json as _json  # noqa: E402
import concourse.bass_utils as _bu  # noqa: E402
from concourse import bass2jax as _b2j  # noqa: E402

_orig_compile_bir_kernel = _bu.compile_bir_kernel


def _lower_multiwait_sync(bir_bytes):
    bir = _json.loads(bir_bytes)
    nsplit = 0
    for fn in bir.get("functions", []):
        for blk in fn.get("blocks", []):
            out = []
            for ins in blk["instructions"]:
                si = ins.get("sync_info")
                waits = (si or {}).get("on_wait") or []
                if len(waits) > 1:
                    for i, w in enumerate(waits[:-1]):
                        nsplit += 1
                        out.append({
                            "debug": ins.get("debug", 0),
                            "engine": ins["engine"],
                            "ins": [],
                            "outs": [],
                            "name": f"{ins['name']}w{i}",
                            "opcode": "EventSemaphore",
                            "sync_info": {"on_wait": [w], "on_update": []},
                        })
                    si["on_wait"] = [waits[-1]]
                out.append(ins)
            blk["instructions"] = out
    return _json.dumps(bir).encode(), nsplit


def _patched_compile_bir_kernel(bir_json, tmpdir, neff_name="file.neff"):
    bir_json, nsplit = _lower_multiwait_sync(bir_json)
    if nsplit:
        print(f"[kernel] split {nsplit} extra sync waits into standalone "
              f"EventSemaphore instructions", flush=True)
    return _orig_compile_bir_kernel(bir_json, tmpdir, neff_name)


_bu.compile_bir_kernel = _patched_compile_bir_kernel
_b2j.compile_bir_kernel = _patched_compile_bir_kernel

# ---------------------------------------------------------------------------
# NTFF profiling under axon: bass_utils wants antenv.axon_hooks (absent in
# this image) whose hook drives axon_{start,stop}_nrt_profile in
# libaxon_pjrt.so.  Recreate that shim here so trace=True works.
# ---------------------------------------------------------------------------
import contextlib as _contextlib  # noqa: E402
import ctypes as _ctypes  # noqa: E402
import types as _types  # noqa: E402

_AXON_SO = "/opt/axon/libaxon_pjrt.so"


def _make_ntff_hook():
    try:
        lib = _ctypes.CDLL(_AXON_SO)
    except OSError:
        return None
    if not hasattr(lib, "axon_start_nrt_profile"):
        return None
    lib.axon_start_nrt_profile.argtypes = [
        _ctypes.POINTER(_ctypes.c_int64), _ctypes.c_size_t]
    lib.axon_start_nrt_profile.restype = _ctypes.c_int64
    lib.axon_stop_nrt_profile.argtypes = [_ctypes.c_char_p]
    lib.axon_stop_nrt_profile.restype = _ctypes.c_int64

    @_contextlib.contextmanager
    def _hook(output_dir, device_ids):
        import jax

        jax.devices()  # force PJRT init so GLOBAL_CLIENT exists
        if device_ids:
            ids = (_ctypes.c_int64 * len(device_ids))(*device_ids)
            rc = lib.axon_start_nrt_profile(ids, len(device_ids))
        else:
            rc = lib.axon_start_nrt_profile(None, 0)
        if rc != 0:
            raise RuntimeError(f"axon_start_nrt_profile rc={rc}")
        try:
            yield
        finally:
            n = lib.axon_stop_nrt_profile(str(output_dir).encode())
            print(f"[kernel] ntff profile: {n} file(s) -> {output_dir}", flush=True)

    return _hook


if "antenv.axon_hooks" not in sys.modules:
    _m = _types.ModuleType("antenv.axon_hooks")
    _m.get_axon_ntff_profile_hook = _make_ntff_hook
    _m.set_axon_ntff_profile_hook = lambda h: None
    sys.modules["antenv.axon_hooks"] = _m

# the artifact upload wants a remote bucket; irrelevant here
_bu.upload_artifacts = lambda tmpdir: f"local:{tmpdir}"

P = 128
D = 1024  # model dim
S = 2048  # full sequence (keys per batch)
M = 1024  # queries per core (half a sequence)
H = 16  # heads
DH = 64  # head dim
NCH = D // P  # 8 feature chunks of 128
FP = mybir.dt.float32
FR = mybir.dt.float32r  # fast fp32 matmul mode (E8M11)
BF = mybir.dt.bfloat16  # attention-core matmul dtype (1 cyc/row + FWL)

Exp = mybir.ActivationFunctionType.Exp
Ident = mybir.ActivationFunctionType.Identity
SCALE = 1.0 / np.sqrt(DH)

_CACHED = {}


def _r3(ap):
    """[ (c p), f ] dram view -> [p, c, f]"""
    return ap.rearrange("(c p) f -> p c f", p=P)


def build_program(mm_dt=FR):
    nc = bass.Bass()
    MD = mm_dt
    xT = nc.dram_tensor("xT", [D, S], MD, kind="ExternalInput")
    xTq = nc.dram_tensor("xTq", [D, M], MD, kind="ExternalInput")
    xTr = nc.dram_tensor("xTr", [D, M], FP, kind="ExternalInput")  # residual
    WqT = nc.dram_tensor("WqT", [D, D], MD, kind="ExternalInput")
    WkT = nc.dram_tensor("WkT", [D, D], MD, kind="ExternalInput")
    WvT = nc.dram_tensor("WvT", [D, D], MD, kind="ExternalInput")
    WoT = nc.dram_tensor("WoT", [D, D], MD, kind="ExternalInput")
    bq = nc.dram_tensor("bq", [D], FP, kind="ExternalInput")
    bk = nc.dram_tensor("bk", [D], FP, kind="ExternalInput")
    bv = nc.dram_tensor("bv", [D], FP, kind="ExternalInput")
    bo = nc.dram_tensor("bo", [D], FP, kind="ExternalInput")
    yT = nc.dram_tensor("yT", [D, M], FP, kind="ExternalOutput")
    v_spill = nc.dram_tensor("v_spill", [H, S, DH], BF)

    def mm(ps, lhsT, rhs, start, stop):
        nc.tensor.matmul(ps, lhsT=lhsT, rhs=rhs, start=start, stop=stop)

    with tile.TileContext(nc) as tc, ExitStack() as ctx:
        ctx.enter_context(
            nc.allow_low_precision(reason="float32r (E8M11) matmul operands by design")
        )
        kq = ctx.enter_context(tc.tile_pool(name="kq", bufs=1))
        K_sb = kq.tile([P, NCH, S], BF, tag="K")  # K.T (bf16)  32KB/part
        Q_sb = kq.tile([P, NCH, M], BF, tag="Q")  # Q.T (bf16)  16KB/part
        bq_sb = kq.tile([P, NCH], FP, tag="bq")
        bk_sb = kq.tile([P, NCH], FP, tag="bk")
        bv_sb = kq.tile([P, NCH], FP, tag="bv")
        bo_sb = kq.tile([P, NCH], FP, tag="bo")
        for t, d in ((bq_sb, bq), (bk_sb, bk), (bv_sb, bv), (bo_sb, bo)):
            nc.sync.dma_start(t[:], d.rearrange("(c p) -> p c", p=P))

        xT3, xTq3, xTr3 = _r3(xT), _r3(xTq), _r3(xTr)

        # ---------------- phase 1: projections ----------------
        with (
            tc.tile_pool(name="w", bufs=1) as wp,
            tc.tile_pool(name="xs", bufs=2) as xp,
            tc.tile_pool(name="pp", bufs=4, space="PSUM") as pp,
            tc.tile_pool(name="vb", bufs=3) as vbp,
        ):
            # --- K.T = WkT.T @ xT (+bk), feature-major [nc, j] ---
            w = wp.tile([P, NCH, D], MD, tag="w")
            nc.sync.dma_start(w[:], _r3(WkT))
            for jb in range(S // 512):
                xs = xp.tile([P, NCH, 512], MD, tag="xs")
                nc.sync.dma_start(xs[:], xT3[:, :, jb * 512 : (jb + 1) * 512])
                for ncx in range(NCH):
                    ps = pp.tile([P, 512], FP, tag="pp")
                    for dc in range(NCH):
                        mm(ps[:], w[:, dc, ncx * P : (ncx + 1) * P], xs[:, dc, :],
                           dc == 0, dc == NCH - 1)
                    nc.scalar.activation(
                        K_sb[:, ncx, jb * 512 : (jb + 1) * 512], ps[:], Ident,
                        bias=bk_sb[:, ncx : ncx + 1],
                    )
            # --- V = xT-chunk.T @ WvT, natural [j, n]; spilled to DRAM ---
            w = wp.tile([P, NCH, D], MD, tag="w")
            nc.sync.dma_start(w[:], _r3(WvT))
            vd = v_spill.rearrange("h (jc p) d -> p jc h d", p=P)
            for jb in range(S // 512):
                xs = xp.tile([P, NCH, 512], MD, tag="xs")
                nc.sync.dma_start(xs[:], xT3[:, :, jb * 512 : (jb + 1) * 512])
                for js in range(4):  # 4 j-chunks of 128 inside the 512 block
                    jc = jb * 4 + js
                    for nh in range(2):  # n halves of 512 (8 heads each)
                        ps = pp.tile([P, 512], FP, tag="pp")
                        for dc in range(NCH):
                            mm(ps[:], xs[:, dc, js * P : (js + 1) * P],
                               w[:, dc, nh * 512 : (nh + 1) * 512], dc == 0, dc == NCH - 1)
                        vb = vbp.tile([P, 512], BF, tag="vb")
                        nc.vector.tensor_copy(vb[:], ps[:])
                        nc.sync.dma_start(vd[:, jc, nh * 8 : (nh + 1) * 8, :], vb[:])
            # --- Q.T (+bq), query half only ---
            w = wp.tile([P, NCH, D], MD, tag="w")
            nc.sync.dma_start(w[:], _r3(WqT))
            for jb in range(M // 512):
                xs = xp.tile([P, NCH, 512], MD, tag="xs")
                nc.sync.dma_start(xs[:], xTq3[:, :, jb * 512 : (jb + 1) * 512])
                for ncx in range(NCH):
                    ps = pp.tile([P, 512], FP, tag="pp")
                    for dc in range(NCH):
                        mm(ps[:], w[:, dc, ncx * P : (ncx + 1) * P], xs[:, dc, :],
                           dc == 0, dc == NCH - 1)
                    nc.scalar.activation(
                        Q_sb[:, ncx, jb * 512 : (jb + 1) * 512], ps[:], Ident,
                        bias=bq_sb[:, ncx : ncx + 1],
                    )

        # ---------------- phase 2: attention ----------------
        with ExitStack() as octx:
            aop = octx.enter_context(tc.tile_pool(name="aop", bufs=1))
            AO_sb = aop.tile([P, NCH, M], MD, tag="AO")  # attn out, feature-major
            # fp32r tiles must be written by "rounding" ops (DVE), not memset
            ones_fp = aop.tile([P, S // P, 2], FP, tag="ones_fp")
            nc.any.memset(ones_fp[:], 1.0)
            ones1_fp = aop.tile([1, DH], FP, tag="ones1_fp")
            nc.any.memset(ones1_fp[:], 1.0)
            ones_sb = aop.tile([1, DH], MD, tag="ones")
            nc.vector.tensor_copy(ones_sb[:], ones1_fp[:])

            with (
                tc.tile_pool(name="vp", bufs=2) as vp,
                tc.tile_pool(name="ptp", bufs=2) as ptp,
                tc.tile_pool(name="sps", bufs=2, space="PSUM") as sps,
                tc.tile_pool(name="pvs", bufs=1, space="PSUM") as pvs,
                tc.tile_pool(name="rbs", bufs=1, space="PSUM") as rbs,
                tc.tile_pool(name="smal", bufs=4) as smal,
            ):
                MB = 512
                for h in range(H):
                    nch, hp = h // 2, (h % 2) * DH
                    v = vp.tile([P, S // P, DH + 2], BF, tag="v")
                    nc.vector.tensor_copy(v[:, :, DH : DH + 2], ones_fp[:])
                    nc.sync.dma_start(
                        v[:, :, 0:DH],
                        v_spill[h].rearrange("(jc p) d -> p jc d", p=P),
                    )
                    for mb in range(M // MB):
                        ms = slice(mb * MB, (mb + 1) * MB)
                        pt = ptp.tile([P, S // P, MB], BF, tag="pt")
                        for g in range(8):  # groups of 2 j-chunks
                            sp = sps.tile([P, 1024], FP, tag="sp")
                            for q in range(2):
                                jc = g * 2 + q
                                mm(sp[:, q * MB : (q + 1) * MB],
                                   K_sb[hp : hp + DH, nch, jc * P : (jc + 1) * P],
                                   Q_sb[hp : hp + DH, nch, ms], True, True)
                            nc.scalar.activation(
                                pt[:, g * 2 : (g + 1) * 2, :].rearrange("p a b -> p (a b)"),
                                sp[:], Exp, scale=float(SCALE),
                            )
                        pv = pvs.tile([P, MB], FP, tag="pv")
                        for jc in range(S // P):
                            mm(pv[0 : DH + 1, :], v[:, jc, 0 : DH + 1], pt[:, jc, :],
                               jc == 0, jc == S // P - 1)
                        pvsb = smal.tile([DH + 1, MB], FP, tag="pvsb")
                        nc.vector.tensor_copy(pvsb[:], pv[0 : DH + 1, :])
                        r = smal.tile([1, MB], MD, tag="r")
                        nc.vector.reciprocal(r[:], pvsb[DH : DH + 1, :])
                        # broadcast 1/r over DH partitions via a K=1 matmul
                        rb = rbs.tile([DH, MB], FP, tag="rb")
                        mm(rb[:], ones_sb[:, 0:DH], r[:], True, True)
                        dst = AO_sb[hp : hp + DH, nch, ms]
                        nc.vector.tensor_mul(dst, pvsb[0:DH, :], rb[:])
                        nc.vector.tensor_scalar_add(
                            dst, dst, bv_sb[hp : hp + DH, nch : nch + 1])

            # ---------------- phase 3: output projection ----------------
            with (
                tc.tile_pool(name="wo", bufs=1) as wop,
                tc.tile_pool(name="op", bufs=4, space="PSUM") as op,
                tc.tile_pool(name="yt", bufs=3) as ytp,
                tc.tile_pool(name="res", bufs=3) as resp,
            ):
                wo = wop.tile([P, NCH, D], MD, tag="wo")
                nc.sync.dma_start(wo[:], _r3(WoT))
                yT3 = _r3(yT)
                for ncx in range(NCH):
                    for mh in range(M // 512):
                        ms = slice(mh * 512, (mh + 1) * 512)
                        ps = op.tile([P, 512], FP, tag="op")
                        for qc in range(NCH):
                            mm(ps[:], wo[:, qc, ncx * P : (ncx + 1) * P],
                               AO_sb[:, qc, ms], qc == 0, qc == NCH - 1)
                        yt = ytp.tile([P, 512], FP, tag="yt")
                        nc.scalar.activation(yt[:], ps[:], Ident,
                                             bias=bo_sb[:, ncx : ncx + 1])
                        res = resp.tile([P, 512], FP, tag="res")
                        nc.sync.dma_start(res[:], xTr3[:, ncx, ms])
                        nc.vector.tensor_add(yt[:], yt[:], res[:])
                        nc.sync.dma_start(yT3[:, ncx, ms], yt[:])
    return nc


def _round_fp32r(a):
    """Round fp32 array to E8M11 (fp32r) with round-to-nearest-even."""
    b = np.ascontiguousarray(a, np.float32).view(np.uint32)
    lsb = (b >> np.uint32(12)) & np.uint32(1)
    r = (b + np.uint32(0x7FF) + lsb) & np.uint32(0xFFFFF000)
    return r.view(np.float32)


def _prep_inputs(x, Wq, bq, Wk, bk, Wv, bv, Wo, bo, round_r=True):
    f32 = np.float32
    rnd = _round_fp32r if round_r else (lambda a: np.ascontiguousarray(a, f32))
    WqT = rnd(np.asarray(Wq, f32).T)
    WkT = rnd(np.asarray(Wk, f32).T)
    WvT = rnd(np.asarray(Wv, f32).T)
    WoT = rnd(np.asarray(Wo, f32).T)
    bq, bk, bv, bo = (np.ascontiguousarray(np.asarray(a, f32)) for a in (bq, bk, bv, bo))
    in_maps = []
    for c in range(8):
        b, half = c // 2, c % 2
        xTb = np.ascontiguousarray(np.asarray(x[b], f32).T)  # [D, S]
        xTq = xTb[:, half * M : (half + 1) * M]
        in_maps.append({
            "xT": rnd(xTb),
            "xTq": rnd(xTq),
            "xTr": np.ascontiguousarray(xTq),
            "WqT": WqT, "WkT": WkT, "WvT": WvT, "WoT": WoT,
            "bq": bq, "bk": bk, "bv": bv, "bo": bo,
        })
    return in_maps


def run(inputs, trace=False, mm_dt=FR):
    key = str(mm_dt)
    if key not in _CACHED:
        _CACHED[key] = build_program(mm_dt)
    nc = _CACHED[key]
    in_maps = _prep_inputs(**inputs, round_r=(mm_dt == FR))
    exec_ns = None
    prof_info = None
    res = run_bass_kernel_spmd(nc, in_maps, list(range(8)), trace=trace)
    results = res.results
    if trace:
        exec_ns = res.exec_time_ns
        prof_info = res.profile_json
    out = np.empty((4, S, D), np.float32)
    for c in range(8):
        b, half = c // 2, c % 2
        out[b, half * M : (half + 1) * M, :] = results[c]["yT"].T
    return out, exec_ns, prof_info


def kernel(**inputs):
    out, _, _ = run(inputs, trace=False)
    return out


# revision 36
# speedup vs baseline: 1.0326x; 1.0326x over previous
"""Trainium2 Bass kernel for an attention block (dense transformer).

Reference computation (per batch b):
    q = x @ Wq.T + bq ; k = x @ Wk.T + bk ; v = x @ Wv.T + bv
    per head: attn = softmax(q k^T / sqrt(dh)) ; o = attn @ v
    out = concat(o) @ Wo.T + bo + x

Sharding: 8 cores = 4 batches x 2 query-halves (data parallel; K/V
projections duplicated within a pair, which avoids all collectives).

Device-side layouts are feature-major ("transposed"): the host passes
x[b].T and W.T so no on-device fp32 transposes are ever needed.
Matmuls run in float32r (fp32 rounded to E8M11, full-rate on the PE —
4x faster than plain fp32). Matmul operands coming from DRAM are
pre-rounded to fp32r on the host; operands produced on-device are
written with float32r output dtype by ACT/DVE so the BIR verifier's
"rounded producer" rule is satisfied.

Softmax: scoresT[j, m] = K.T-chunk.T @ Q.T, exp on ScalarE (no
max-subtraction needed: |scores| < ~3 for this distribution), and a
ones-column appended to V so P @ [V | 1] yields both P@V and the row
sums in one PSUM accumulation group. bv is folded in after
normalization (attn rows sum to 1).
"""

import os
import sys
from contextlib import ExitStack

import numpy as np

sys.path.insert(0, "/opt/trn_rl_repo")
os.environ.setdefault("MYCRO_LOCAL_CACHE", "1")

import concourse.bass as bass  # noqa: E402
import concourse.tile as tile  # noqa: E402
from concourse import mybir  # noqa: E402
from concourse.bass_utils import run_bass_kernel_spmd  # noqa: E402

# ---------------------------------------------------------------------------
# walrus codegen in this toolchain encodes at most ONE semaphore wait per
# instruction ("Too many sync wait commands").  Tile's scheduler freely emits
# several.  Split every multi-wait sync_info into standalone EventSemaphore
# wait instructions on the same engine, immediately before the instruction —
# semantically identical (engine sequencers execute them in program order).
# ---------------------------------------------------------------------------
import json as _json  # noqa: E402
import concourse.bass_utils as _bu  # noqa: E402
from concourse import bass2jax as _b2j  # noqa: E402

_orig_compile_bir_kernel = _bu.compile_bir_kernel


def _lower_multiwait_sync(bir_bytes):
    bir = _json.loads(bir_bytes)
    nsplit = 0
    for fn in bir.get("functions", []):
        for blk in fn.get("blocks", []):
            out = []
            for ins in blk["instructions"]:
                si = ins.get("sync_info")
                waits = (si or {}).get("on_wait") or []
                if len(waits) > 1:
                    for i, w in enumerate(waits[:-1]):
                        nsplit += 1
                        out.append({
                            "debug": ins.get("debug", 0),
                            "engine": ins["engine"],
                            "ins": [],
                            "outs": [],
                            "name": f"{ins['name']}w{i}",
                            "opcode": "EventSemaphore",
                            "sync_info": {"on_wait": [w], "on_update": []},
                        })
                    si["on_wait"] = [waits[-1]]
                out.append(ins)
            blk["instructions"] = out
    return _json.dumps(bir).encode(), nsplit


def _patched_compile_bir_kernel(bir_json, tmpdir, neff_name="file.neff"):
    bir_json, nsplit = _lower_multiwait_sync(bir_json)
    if nsplit:
        print(f"[kernel] split {nsplit} extra sync waits into standalone "
              f"EventSemaphore instructions", flush=True)
    return _orig_compile_bir_kernel(bir_json, tmpdir, neff_name)


_bu.compile_bir_kernel = _patched_compile_bir_kernel
_b2j.compile_bir_kernel = _patched_compile_bir_kernel

# ---------------------------------------------------------------------------
# NTFF profiling under axon: bass_utils wants antenv.axon_hooks (absent in
# this image) whose hook drives axon_{start,stop}_nrt_profile in
# libaxon_pjrt.so.  Recreate that shim here so trace=True works.
# ---------------------------------------------------------------------------
import contextlib as _contextlib  # noqa: E402
import ctypes as _ctypes  # noqa: E402
import types as _types  # noqa: E402

_AXON_SO = "/opt/axon/libaxon_pjrt.so"


def _make_ntff_hook():
    try:
        lib = _ctypes.CDLL(_AXON_SO)
    except OSError:
        return None
    if not hasattr(lib, "axon_start_nrt_profile"):
        return None
    lib.axon_start_nrt_profile.argtypes = [
        _ctypes.POINTER(_ctypes.c_int64), _ctypes.c_size_t]
    lib.axon_start_nrt_profile.restype = _ctypes.c_int64
    lib.axon_stop_nrt_profile.argtypes = [_ctypes.c_char_p]
    lib.axon_stop_nrt_profile.restype = _ctypes.c_int64

    @_contextlib.contextmanager
    def _hook(output_dir, device_ids):
        import jax

        jax.devices()  # force PJRT init so GLOBAL_CLIENT exists
        if device_ids:
            ids = (_ctypes.c_int64 * len(device_ids))(*device_ids)
            rc = lib.axon_start_nrt_profile(ids, len(device_ids))
        else:
            rc = lib.axon_start_nrt_profile(None, 0)
        if rc != 0:
            raise RuntimeError(f"axon_start_nrt_profile rc={rc}")
        try:
            yield
        finally:
            n = lib.axon_stop_nrt_profile(str(output_dir).encode())
            print(f"[kernel] ntff profile: {n} file(s) -> {output_dir}", flush=True)

    return _hook


if "antenv.axon_hooks" not in sys.modules:
    _m = _types.ModuleType("antenv.axon_hooks")
    _m.get_axon_ntff_profile_hook = _make_ntff_hook
    _m.set_axon_ntff_profile_hook = lambda h: None
    sys.modules["antenv.axon_hooks"] = _m

# the artifact upload wants a remote bucket; irrelevant here
_bu.upload_artifacts = lambda tmpdir: f"local:{tmpdir}"

P = 128
D = 1024  # model dim
S = 2048  # full sequence (keys per batch)
M = 1024  # queries per core (half a sequence)
H = 16  # heads
DH = 64  # head dim
NCH = D // P  # 8 feature chunks of 128
FP = mybir.dt.float32
FR = mybir.dt.float32r  # fast fp32 matmul mode (E8M11)
BF = mybir.dt.bfloat16  # attention-core matmul dtype (1 cyc/row + FWL)

Exp = mybir.ActivationFunctionType.Exp
Ident = mybir.ActivationFunctionType.Identity
SCALE = 1.0 / np.sqrt(DH)

_CACHED = {}


def _r3(ap):
    """[ (c p), f ] dram view -> [p, c, f]"""
    return ap.rearrange("(c p) f -> p c f", p=P)


def build_program(mm_dt=FR):
    nc = bass.Bass()
    MD = mm_dt
    xT = nc.dram_tensor("xT", [D, S], MD, kind="ExternalInput")
    xTq = nc.dram_tensor("xTq", [D, M], MD, kind="ExternalInput")
    xTr = nc.dram_tensor("xTr", [D, M], FP, kind="ExternalInput")  # residual
    WqT = nc.dram_tensor("WqT", [D, D], MD, kind="ExternalInput")
    WkT = nc.dram_tensor("WkT", [D, D], MD, kind="ExternalInput")
    WvT = nc.dram_tensor("WvT", [D, D], MD, kind="ExternalInput")
    WoT = nc.dram_tensor("WoT", [D, D], MD, kind="ExternalInput")
    bq = nc.dram_tensor("bq", [D], FP, kind="ExternalInput")
    bk = nc.dram_tensor("bk", [D], FP, kind="ExternalInput")
    bv = nc.dram_tensor("bv", [D], FP, kind="ExternalInput")
    bo = nc.dram_tensor("bo", [D], FP, kind="ExternalInput")
    yT = nc.dram_tensor("yT", [D, M], FP, kind="ExternalOutput")
    v_spill = nc.dram_tensor("v_spill", [H, S, DH], BF)

    def mm(ps, lhsT, rhs, start, stop):
        nc.tensor.matmul(ps, lhsT=lhsT, rhs=rhs, start=start, stop=stop)

    with tile.TileContext(nc) as tc, ExitStack() as ctx:
        ctx.enter_context(
            nc.allow_low_precision(reason="float32r (E8M11) matmul operands by design")
        )
        kq = ctx.enter_context(tc.tile_pool(name="kq", bufs=1))
        K_sb = kq.tile([P, NCH, S], BF, tag="K")  # K.T (bf16)  32KB/part
        Q_sb = kq.tile([P, NCH, M], BF, tag="Q")  # Q.T (bf16)  16KB/part
        bq_sb = kq.tile([P, NCH], FP, tag="bq")
        bk_sb = kq.tile([P, NCH], FP, tag="bk")
        bv_sb = kq.tile([P, NCH], FP, tag="bv")
        bo_sb = kq.tile([P, NCH], FP, tag="bo")
        for t, d in ((bq_sb, bq), (bk_sb, bk), (bv_sb, bv), (bo_sb, bo)):
            nc.sync.dma_start(t[:], d.rearrange("(c p) -> p c", p=P))

        xT3, xTq3, xTr3 = _r3(xT), _r3(xTq), _r3(xTr)

        # ---------------- phase 1: projections ----------------
        with (
            tc.tile_pool(name="w", bufs=1) as wp,
            tc.tile_pool(name="xs", bufs=2) as xp,
            tc.tile_pool(name="pp", bufs=4, space="PSUM") as pp,
            tc.tile_pool(name="vb", bufs=3) as vbp,
        ):
            # --- K.T = WkT.T @ xT (+bk), feature-major [nc, j] ---
            w = wp.tile([P, NCH, D], MD, tag="w")
            nc.sync.dma_start(w[:], _r3(WkT))
            for jb in range(S // 512):
                xs = xp.tile([P, NCH, 512], MD, tag="xs")
                nc.sync.dma_start(xs[:], xT3[:, :, jb * 512 : (jb + 1) * 512])
                for ncx in range(NCH):
                    ps = pp.tile([P, 512], FP, tag="pp")
                    for dc in range(NCH):
                        mm(ps[:], w[:, dc, ncx * P : (ncx + 1) * P], xs[:, dc, :],
                           dc == 0, dc == NCH - 1)
                    nc.vector.tensor_scalar_add(
                        K_sb[:, ncx, jb * 512 : (jb + 1) * 512], ps[:],
                        bk_sb[:, ncx : ncx + 1])
            # --- V = xT-chunk.T @ WvT, natural [j, n]; spilled to DRAM ---
            w = wp.tile([P, NCH, D], MD, tag="w")
            nc.sync.dma_start(w[:], _r3(WvT))
            vd = v_spill.rearrange("h (jc p) d -> p jc h d", p=P)
            for jb in range(S // 512):
                xs = xp.tile([P, NCH, 512], MD, tag="xs")
                nc.sync.dma_start(xs[:], xT3[:, :, jb * 512 : (jb + 1) * 512])
                for js in range(4):  # 4 j-chunks of 128 inside the 512 block
                    jc = jb * 4 + js
                    for nh in range(2):  # n halves of 512 (8 heads each)
                        ps = pp.tile([P, 512], FP, tag="pp")
                        for dc in range(NCH):
                            mm(ps[:], xs[:, dc, js * P : (js + 1) * P],
                               w[:, dc, nh * 512 : (nh + 1) * 512], dc == 0, dc == NCH - 1)
                        vb = vbp.tile([P, 512], BF, tag="vb")
                        nc.vector.tensor_copy(vb[:], ps[:])
                        nc.sync.dma_start(vd[:, jc, nh * 8 : (nh + 1) * 8, :], vb[:])
            # --- Q.T (+bq), query half only ---
            w = wp.tile([P, NCH, D], MD, tag="w")
            nc.sync.dma_start(w[:], _r3(WqT))
            for jb in range(M // 512):
                xs = xp.tile([P, NCH, 512], MD, tag="xs")
                nc.sync.dma_start(xs[:], xTq3[:, :, jb * 512 : (jb + 1) * 512])
                for ncx in range(NCH):
                    ps = pp.tile([P, 512], FP, tag="pp")
                    for dc in range(NCH):
                        mm(ps[:], w[:, dc, ncx * P : (ncx + 1) * P], xs[:, dc, :],
                           dc == 0, dc == NCH - 1)
                    nc.vector.tensor_scalar_add(
                        Q_sb[:, ncx, jb * 512 : (jb + 1) * 512], ps[:],
                        bq_sb[:, ncx : ncx + 1])

        # ---------------- phase 2: attention ----------------
        with ExitStack() as octx:
            aop = octx.enter_context(tc.tile_pool(name="aop", bufs=1))
            AO_sb = aop.tile([P, NCH, M], MD, tag="AO")  # attn out, feature-major
            # fp32r tiles must be written by "rounding" ops (DVE), not memset
            ones_fp = aop.tile([P, S // P, 2], FP, tag="ones_fp")
            nc.any.memset(ones_fp[:], 1.0)
            ones1_fp = aop.tile([1, DH], FP, tag="ones1_fp")
            nc.any.memset(ones1_fp[:], 1.0)
            ones_sb = aop.tile([1, DH], MD, tag="ones")
            nc.vector.tensor_copy(ones_sb[:], ones1_fp[:])

            with (
                tc.tile_pool(name="vp", bufs=2) as vp,
                tc.tile_pool(name="ptp", bufs=2) as ptp,
                tc.tile_pool(name="sps", bufs=2, space="PSUM") as sps,
                tc.tile_pool(name="pvs", bufs=2, space="PSUM") as pvs,
                tc.tile_pool(name="rbs", bufs=2, space="PSUM") as rbs,
                tc.tile_pool(name="smal", bufs=4) as smal,
            ):
                MB = 512
                # 2-deep software pipeline over (head, m-block) iterations:
                #   stage A (iter i):   score matmuls + exp, with the PREVIOUS
                #                       iteration's PV matmuls interleaved so
                #                       the PE never idles while ACT exps
                #   stage B (iter i):   pvsb copy + reciprocal of iter i-1
                #                       (DVE; finishes during iter i+1)
                #   stage C (iter i):   broadcast-matmul + normalize of i-2
                #                       (reciprocal latency fully hidden)
                items = [(h, mb) for h in range(H) for mb in range(M // MB)]
                state = {}  # it -> dict

                def stage_a(i):
                    h, mb = items[i]
                    nch, hp = h // 2, (h % 2) * DH
                    ms = slice(mb * MB, (mb + 1) * MB)
                    if mb == 0:
                        v = vp.tile([P, S // P, DH + 2], BF, tag="v")
                        nc.vector.tensor_copy(v[:, :, DH : DH + 2], ones_fp[:])
                        nc.sync.dma_start(
                            v[:, :, 0:DH],
                            v_spill[h].rearrange("(jc p) d -> p jc d", p=P),
                        )
                    else:
                        v = state[(i - 1)]["v"]
                    pt = ptp.tile([P, S // P, MB], BF, tag="pt")
                    st = dict(h=h, nch=nch, hp=hp, ms=ms, v=v, pt=pt, pv=None)
                    state[i] = st
                    prev = state.get(i - 1)
                    if prev is not None:
                        prev["pv"] = pvs.tile([P, MB], FP, tag="pv", name="pv")
                    for g in range(8):  # groups of 2 j-chunks
                        sp = sps.tile([P, 1024], FP, tag="sp")
                        for q in range(2):
                            jc = g * 2 + q
                            mm(sp[:, q * MB : (q + 1) * MB],
                               K_sb[hp : hp + DH, nch, jc * P : (jc + 1) * P],
                               Q_sb[hp : hp + DH, nch, ms], True, True)
                        nc.scalar.activation(
                            pt[:, g * 2 : (g + 1) * 2, :].rearrange("p a b -> p (a b)"),
                            sp[:], Exp, scale=float(SCALE),
                        )
                        if prev is not None:
                            for jc in (2 * g, 2 * g + 1):
                                mm(prev["pv"][0 : DH + 1, :],
                                   prev["v"][:, jc, 0 : DH + 1],
                                   prev["pt"][:, jc, :],
                                   jc == 0, jc == S // P - 1)

                def stage_b(i):  # pvsb copy + reciprocal for iter i
                    st = state[i]
                    pvsb = smal.tile([DH + 1, MB], FP, tag="pvsb")
                    nc.vector.tensor_copy(pvsb[:], st["pv"][0 : DH + 1, :])
                    r = smal.tile([1, MB], MD, tag="r")
                    nc.vector.reciprocal(r[:], pvsb[DH : DH + 1, :])
                    st["pvsb"], st["r"] = pvsb, r

                def stage_c(i):  # broadcast + normalize for iter i
                    st = state[i]
                    rb = rbs.tile([DH, MB], FP, tag="rb")
                    mm(rb[:], ones_sb[:, 0:DH], st["r"][:], True, True)
                    dst = AO_sb[st["hp"] : st["hp"] + DH, st["nch"], st["ms"]]
                    nc.vector.tensor_mul(dst, st["pvsb"][0:DH, :], rb[:])
                    nc.vector.tensor_scalar_add(
                        dst, dst, bv_sb[st["hp"] : st["hp"] + DH,
                                        st["nch"] : st["nch"] + 1])
                    del state[i]

                N_IT = len(items)
                for i in range(N_IT):
                    stage_a(i)
                    if i >= 1:
                        stage_b(i - 1)
                    if i >= 2:
                        stage_c(i - 2)
                # drain: PV of the last iteration, then remaining tails
                last = N_IT - 1
                state[last]["pv"] = pvs.tile([P, MB], FP, tag="pv", name="pv")
                for jc in range(S // P):
                    mm(state[last]["pv"][0 : DH + 1, :],
                       state[last]["v"][:, jc, 0 : DH + 1],
                       state[last]["pt"][:, jc, :],
                       jc == 0, jc == S // P - 1)
                stage_b(last)
                stage_c(last - 1)
                stage_c(last)

            # ---------------- phase 3: output projection ----------------
            with (
                tc.tile_pool(name="wo", bufs=1) as wop,
                tc.tile_pool(name="op", bufs=4, space="PSUM") as op,
                tc.tile_pool(name="yt", bufs=3) as ytp,
                tc.tile_pool(name="res", bufs=3) as resp,
            ):
                wo = wop.tile([P, NCH, D], MD, tag="wo")
                nc.sync.dma_start(wo[:], _r3(WoT))
                yT3 = _r3(yT)
                for ncx in range(NCH):
                    for mh in range(M // 512):
                        ms = slice(mh * 512, (mh + 1) * 512)
                        ps = op.tile([P, 512], FP, tag="op")
                        for qc in range(NCH):
                            mm(ps[:], wo[:, qc, ncx * P : (ncx + 1) * P],
                               AO_sb[:, qc, ms], qc == 0, qc == NCH - 1)
                        yt = ytp.tile([P, 512], FP, tag="yt")
                        nc.scalar.activation(yt[:], ps[:], Ident,
                                             bias=bo_sb[:, ncx : ncx + 1])
                        res = resp.tile([P, 512], FP, tag="res")
                        nc.sync.dma_start(res[:], xTr3[:, ncx, ms])
                        nc.vector.tensor_add(yt[:], yt[:], res[:])
                        nc.sync.dma_start(yT3[:, ncx, ms], yt[:])
    return nc


def _round_fp32r(a):
    """Round fp32 array to E8M11 (fp32r) with round-to-nearest-even."""
    b = np.ascontiguousarray(a, np.float32).view(np.uint32)
    lsb = (b >> np.uint32(12)) & np.uint32(1)
    r = (b + np.uint32(0x7FF) + lsb) & np.uint32(0xFFFFF000)
    return r.view(np.float32)


def _prep_inputs(x, Wq, bq, Wk, bk, Wv, bv, Wo, bo, round_r=True):
    f32 = np.float32
    rnd = _round_fp32r if round_r else (lambda a: np.ascontiguousarray(a, f32))
    WqT = rnd(np.asarray(Wq, f32).T)
    WkT = rnd(np.asarray(Wk, f32).T)
    WvT = rnd(np.asarray(Wv, f32).T)
    WoT = rnd(np.asarray(Wo, f32).T)
    bq, bk, bv, bo = (np.ascontiguousarray(np.asarray(a, f32)) for a in (bq, bk, bv, bo))
    in_maps = []
    for c in range(8):
        b, half = c // 2, c % 2
        xTb = np.ascontiguousarray(np.asarray(x[b], f32).T)  # [D, S]
        xTq = xTb[:, half * M : (half + 1) * M]
        in_maps.append({
            "xT": rnd(xTb),
            "xTq": rnd(xTq),
            "xTr": np.ascontiguousarray(xTq),
            "WqT": WqT, "WkT": WkT, "WvT": WvT, "WoT": WoT,
            "bq": bq, "bk": bk, "bv": bv, "bo": bo,
        })
    return in_maps


def run(inputs, trace=False, mm_dt=FR):
    key = str(mm_dt)
    if key not in _CACHED:
        _CACHED[key] = build_program(mm_dt)
    nc = _CACHED[key]
    in_maps = _prep_inputs(**inputs, round_r=(mm_dt == FR))
    exec_ns = None
    prof_info = None
    res = run_bass_kernel_spmd(nc, in_maps, list(range(8)), trace=trace)
    results = res.results
    if trace:
        exec_ns = res.exec_time_ns
        prof_info = res.profile_json
    out = np.empty((4, S, D), np.float32)
    for c in range(8):
        b, half = c // 2, c % 2
        out[b, half * M : (half + 1) * M, :] = results[c]["yT"].T
    return out, exec_ns, prof_info


def kernel(**inputs):
    out, _, _ = run(inputs, trace=False)
    return out


# revision 37
# speedup vs baseline: 1.1610x; 1.1243x over previous
"""Trainium2 Bass kernel for an attention block (dense transformer).

Reference computation (per batch b):
    q = x @ Wq.T + bq ; k = x @ Wk.T + bk ; v = x @ Wv.T + bv
    per head: attn = softmax(q k^T / sqrt(dh)) ; o = attn @ v
    out = concat(o) @ Wo.T + bo + x

Sharding: 8 cores = 4 batches x 2 query-halves (data parallel; K/V
projections duplicated within a pair, which avoids all collectives).

Device-side layouts are feature-major ("transposed"): the host passes
x[b].T and W.T so no on-device fp32 transposes are ever needed.
Matmuls run in float32r (fp32 rounded to E8M11, full-rate on the PE —
4x faster than plain fp32). Matmul operands coming from DRAM are
pre-rounded to fp32r on the host; operands produced on-device are
written with float32r output dtype by ACT/DVE so the BIR verifier's
"rounded producer" rule is satisfied.

Softmax: scoresT[j, m] = K.T-chunk.T @ Q.T, exp on ScalarE (no
max-subtraction needed: |scores| < ~3 for this distribution), and a
ones-column appended to V so P @ [V | 1] yields both P@V and the row
sums in one PSUM accumulation group. bv is folded in after
normalization (attn rows sum to 1).
"""

import os
import sys
from contextlib import ExitStack

import numpy as np

sys.path.insert(0, "/opt/trn_rl_repo")
os.environ.setdefault("MYCRO_LOCAL_CACHE", "1")

import concourse.bass as bass  # noqa: E402
import concourse.tile as tile  # noqa: E402
from concourse import mybir  # noqa: E402
from concourse.bass_utils import run_bass_kernel_spmd  # noqa: E402

# ---------------------------------------------------------------------------
# walrus codegen in this toolchain encodes at most ONE semaphore wait per
# instruction ("Too many sync wait commands").  Tile's scheduler freely emits
# several.  Split every multi-wait sync_info into standalone EventSemaphore
# wait instructions on the same engine, immediately before the instruction —
# semantically identical (engine sequencers execute them in program order).
# ---------------------------------------------------------------------------
import json as _json  # noqa: E402
import concourse.bass_utils as _bu  # noqa: E402
from concourse import bass2jax as _b2j  # noqa: E402

_orig_compile_bir_kernel = _bu.compile_bir_kernel


def _lower_multiwait_sync(bir_bytes):
    bir = _json.loads(bir_bytes)
    nsplit = 0
    for fn in bir.get("functions", []):
        for blk in fn.get("blocks", []):
            out = []
            for ins in blk["instructions"]:
                si = ins.get("sync_info")
                waits = (si or {}).get("on_wait") or []
                if len(waits) > 1:
                    for i, w in enumerate(waits[:-1]):
                        nsplit += 1
                        out.append({
                            "debug": ins.get("debug", 0),
                            "engine": ins["engine"],
                            "ins": [],
                            "outs": [],
                            "name": f"{ins['name']}w{i}",
                            "opcode": "EventSemaphore",
                            "sync_info": {"on_wait": [w], "on_update": []},
                        })
                    si["on_wait"] = [waits[-1]]
                out.append(ins)
            blk["instructions"] = out
    return _json.dumps(bir).encode(), nsplit


def _patched_compile_bir_kernel(bir_json, tmpdir, neff_name="file.neff"):
    bir_json, nsplit = _lower_multiwait_sync(bir_json)
    if nsplit:
        print(f"[kernel] split {nsplit} extra sync waits into standalone "
              f"EventSemaphore instructions", flush=True)
    return _orig_compile_bir_kernel(bir_json, tmpdir, neff_name)


_bu.compile_bir_kernel = _patched_compile_bir_kernel
_b2j.compile_bir_kernel = _patched_compile_bir_kernel

# ---------------------------------------------------------------------------
# NTFF profiling under axon: bass_utils wants antenv.axon_hooks (absent in
# this image) whose hook drives axon_{start,stop}_nrt_profile in
# libaxon_pjrt.so.  Recreate that shim here so trace=True works.
# ---------------------------------------------------------------------------
import contextlib as _contextlib  # noqa: E402
import ctypes as _ctypes  # noqa: E402
import types as _types  # noqa: E402

_AXON_SO = "/opt/axon/libaxon_pjrt.so"


def _make_ntff_hook():
    try:
        lib = _ctypes.CDLL(_AXON_SO)
    except OSError:
        return None
    if not hasattr(lib, "axon_start_nrt_profile"):
        return None
    lib.axon_start_nrt_profile.argtypes = [
        _ctypes.POINTER(_ctypes.c_int64), _ctypes.c_size_t]
    lib.axon_start_nrt_profile.restype = _ctypes.c_int64
    lib.axon_stop_nrt_profile.argtypes = [_ctypes.c_char_p]
    lib.axon_stop_nrt_profile.restype = _ctypes.c_int64

    @_contextlib.contextmanager
    def _hook(output_dir, device_ids):
        import jax

        jax.devices()  # force PJRT init so GLOBAL_CLIENT exists
        if device_ids:
            ids = (_ctypes.c_int64 * len(device_ids))(*device_ids)
            rc = lib.axon_start_nrt_profile(ids, len(device_ids))
        else:
            rc = lib.axon_start_nrt_profile(None, 0)
        if rc != 0:
            raise RuntimeError(f"axon_start_nrt_profile rc={rc}")
        try:
            yield
        finally:
            n = lib.axon_stop_nrt_profile(str(output_dir).encode())
            print(f"[kernel] ntff profile: {n} file(s) -> {output_dir}", flush=True)

    return _hook


if "antenv.axon_hooks" not in sys.modules:
    _m = _types.ModuleType("antenv.axon_hooks")
    _m.get_axon_ntff_profile_hook = _make_ntff_hook
    _m.set_axon_ntff_profile_hook = lambda h: None
    sys.modules["antenv.axon_hooks"] = _m

# the artifact upload wants a remote bucket; irrelevant here
_bu.upload_artifacts = lambda tmpdir: f"local:{tmpdir}"

P = 128
D = 1024  # model dim
S = 2048  # full sequence (keys per batch)
M = 1024  # queries per core (half a sequence)
H = 16  # heads
DH = 64  # head dim
NCH = D // P  # 8 feature chunks of 128
FP = mybir.dt.float32
FR = mybir.dt.float32r  # fast fp32 matmul mode (E8M11)
BF = mybir.dt.bfloat16  # attention-core matmul dtype (1 cyc/row + FWL)

Exp = mybir.ActivationFunctionType.Exp
Ident = mybir.ActivationFunctionType.Identity
SCALE = 1.0 / np.sqrt(DH)

_CACHED = {}


def _r3(ap):
    """[ (c p), f ] dram view -> [p, c, f]"""
    return ap.rearrange("(c p) f -> p c f", p=P)


def build_program(mm_dt=FR):
    nc = bass.Bass()
    MD = mm_dt
    xT = nc.dram_tensor("xT", [D, S], MD, kind="ExternalInput")
    xTq = nc.dram_tensor("xTq", [D, M], MD, kind="ExternalInput")
    xTr = nc.dram_tensor("xTr", [D, M], FP, kind="ExternalInput")  # residual
    WqT = nc.dram_tensor("WqT", [D, D], MD, kind="ExternalInput")
    WkT = nc.dram_tensor("WkT", [D, D], MD, kind="ExternalInput")
    WvT = nc.dram_tensor("WvT", [D, D], MD, kind="ExternalInput")
    WoT = nc.dram_tensor("WoT", [D, D], MD, kind="ExternalInput")
    bq = nc.dram_tensor("bq", [D], FP, kind="ExternalInput")
    bk = nc.dram_tensor("bk", [D], FP, kind="ExternalInput")
    bv = nc.dram_tensor("bv", [D], FP, kind="ExternalInput")
    bo = nc.dram_tensor("bo", [D], FP, kind="ExternalInput")
    yT = nc.dram_tensor("yT", [D, M], FP, kind="ExternalOutput")
    v_spill = nc.dram_tensor("v_spill", [H, S, DH], BF)

    def mm(ps, lhsT, rhs, start, stop):
        nc.tensor.matmul(ps, lhsT=lhsT, rhs=rhs, start=start, stop=stop)

    with tile.TileContext(nc) as tc, ExitStack() as ctx:
        ctx.enter_context(
            nc.allow_low_precision(reason="float32r (E8M11) matmul operands by design")
        )
        kq = ctx.enter_context(tc.tile_pool(name="kq", bufs=1))
        K_sb = kq.tile([P, NCH, S], BF, tag="K")  # K.T (bf16)  32KB/part
        Q_sb = kq.tile([P, NCH, M], BF, tag="Q")  # Q.T (bf16)  16KB/part
        bq_sb = kq.tile([P, NCH], FP, tag="bq")
        bk_sb = kq.tile([P, NCH], FP, tag="bk")
        bv_sb = kq.tile([P, NCH], FP, tag="bv")
        bo_sb = kq.tile([P, NCH], FP, tag="bo")
        for t, d in ((bq_sb, bq), (bk_sb, bk), (bv_sb, bv), (bo_sb, bo)):
            nc.sync.dma_start(t[:], d.rearrange("(c p) -> p c", p=P))

        xT3, xTq3, xTr3 = _r3(xT), _r3(xTq), _r3(xTr)

        # ------------------------------------------------------------------
        # Integrated schedule.  HAM throttles the PE array to 1.2 GHz unless
        # it stays nearly continuously busy, and attention alone has less PE
        # work than ScalarE exp work.  So the V-projection's second half is
        # interleaved INTO the attention stream as PE filler:
        #   A: K-proj, Q-proj, V-proj heads 0-7          (dense PE)
        #   B: attention heads 0-7  ||  V-proj heads 8-15 (2 groups/iter)
        #   C: attention heads 8-15
        #   D: output projection
        # Attention itself is a 2-deep software pipeline per (head, m-block):
        # previous iteration's PV matmuls are interleaved between score
        # groups; the reciprocal is consumed two iterations later.
        # ------------------------------------------------------------------
        MB = 512
        with (
            tc.tile_pool(name="w", bufs=1) as wp,
            tc.tile_pool(name="xs", bufs=2) as xp,
            tc.tile_pool(name="pp", bufs=2, space="PSUM") as pp,
            tc.tile_pool(name="vb", bufs=2) as vbp,
            tc.tile_pool(name="aop", bufs=1) as aop,
            tc.tile_pool(name="vp", bufs=2) as vp,
            tc.tile_pool(name="ptp", bufs=2) as ptp,
            tc.tile_pool(name="sps", bufs=2, space="PSUM") as sps,
            tc.tile_pool(name="pvs", bufs=1, space="PSUM") as pvs,
            tc.tile_pool(name="rbs", bufs=1, space="PSUM") as rbs,
            tc.tile_pool(name="smal", bufs=3) as smal,
        ):
            AO_sb = aop.tile([P, NCH, M], MD, tag="AO")  # attn out, feature-major
            ones_fp = aop.tile([P, S // P, 2], FP, tag="ones_fp")
            nc.any.memset(ones_fp[:], 1.0)
            ones1_fp = aop.tile([1, DH], FP, tag="ones1_fp")
            nc.any.memset(ones1_fp[:], 1.0)
            ones_sb = aop.tile([1, DH], MD, tag="ones")
            nc.vector.tensor_copy(ones_sb[:], ones1_fp[:])
            vd = v_spill.rearrange("h (jc p) d -> p jc h d", p=P)

            # --- phase A1: K.T = WkT.T @ xT (+bk), feature-major ---
            w = wp.tile([P, NCH, D], MD, tag="w", name="w")
            nc.sync.dma_start(w[:], _r3(WkT))
            for jb in range(S // 512):
                xs = xp.tile([P, NCH, 512], MD, tag="xs", name="xs")
                nc.sync.dma_start(xs[:], xT3[:, :, jb * 512 : (jb + 1) * 512])
                for ncx in range(NCH):
                    ps = pp.tile([P, 512], FP, tag="pp", name="ps")
                    for dc in range(NCH):
                        mm(ps[:], w[:, dc, ncx * P : (ncx + 1) * P], xs[:, dc, :],
                           dc == 0, dc == NCH - 1)
                    nc.vector.tensor_scalar_add(
                        K_sb[:, ncx, jb * 512 : (jb + 1) * 512], ps[:],
                        bk_sb[:, ncx : ncx + 1])
            # --- phase A2: Q.T (+bq), query half only ---
            w = wp.tile([P, NCH, D], MD, tag="w", name="w")
            nc.sync.dma_start(w[:], _r3(WqT))
            for jb in range(M // 512):
                xs = xp.tile([P, NCH, 512], MD, tag="xs", name="xs")
                nc.sync.dma_start(xs[:], xTq3[:, :, jb * 512 : (jb + 1) * 512])
                for ncx in range(NCH):
                    ps = pp.tile([P, 512], FP, tag="pp", name="ps")
                    for dc in range(NCH):
                        mm(ps[:], w[:, dc, ncx * P : (ncx + 1) * P], xs[:, dc, :],
                           dc == 0, dc == NCH - 1)
                    nc.vector.tensor_scalar_add(
                        Q_sb[:, ncx, jb * 512 : (jb + 1) * 512], ps[:],
                        bq_sb[:, ncx : ncx + 1])
            # --- V = xT-chunk.T @ WvT, natural [j, n]; spilled to DRAM.
            # One half (8 heads) at a time; nh=1 is emitted lazily as PE
            # filler inside the attention stream (phase B).
            wv = wp.tile([P, NCH, D], MD, tag="w", name="w")
            nc.sync.dma_start(wv[:], _r3(WvT))

            def v_group_iter(nh):
                for jb in range(S // 512):
                    xs = xp.tile([P, NCH, 512], MD, tag="xs", name="xs")
                    nc.sync.dma_start(xs[:], xT3[:, :, jb * 512 : (jb + 1) * 512])
                    for js in range(4):
                        yield (xs, jb * 4 + js)

            def emit_v_group(xs, jc, nh):
                ps = pp.tile([P, 512], FP, tag="pp", name="ps")
                js = jc % 4
                for dc in range(NCH):
                    mm(ps[:], xs[:, dc, js * P : (js + 1) * P],
                       wv[:, dc, nh * 512 : (nh + 1) * 512], dc == 0, dc == NCH - 1)
                vbt = vbp.tile([P, 512], BF, tag="vb", name="vbt")
                nc.vector.tensor_copy(vbt[:], ps[:])
                nc.sync.dma_start(vd[:, jc, nh * 8 : (nh + 1) * 8, :], vbt[:])

            for xs, jc in v_group_iter(0):  # phase A3: heads 0-7
                emit_v_group(xs, jc, 0)
            vfill = v_group_iter(1)  # phase B filler: heads 8-15

            # ---------------- attention pipeline ----------------
            items = [(h, mb) for h in range(H) for mb in range(M // MB)]
            state = {}

            def stage_a(i):
                h, mb = items[i]
                nch, hp = h // 2, (h % 2) * DH
                ms = slice(mb * MB, (mb + 1) * MB)
                if mb == 0:
                    v = vp.tile([P, S // P, DH + 2], BF, tag="v", name="v")
                    nc.vector.tensor_copy(v[:, :, DH : DH + 2], ones_fp[:])
                    nc.sync.dma_start(
                        v[:, :, 0:DH],
                        v_spill[h].rearrange("(jc p) d -> p jc d", p=P),
                    )
                else:
                    v = state[i - 1]["v"]
                pt = ptp.tile([P, S // P, MB], BF, tag="pt", name="pt")
                st = dict(h=h, nch=nch, hp=hp, ms=ms, v=v, pt=pt, pv=None)
                state[i] = st
                prev = state.get(i - 1)
                if prev is not None:
                    prev["pv"] = pvs.tile([P, MB], FP, tag="pv", name="pv")
                for g in range(8):  # groups of 2 j-chunks
                    sp = sps.tile([P, 1024], FP, tag="sp", name="sp")
                    for q in range(2):
                        jc = g * 2 + q
                        mm(sp[:, q * MB : (q + 1) * MB],
                           K_sb[hp : hp + DH, nch, jc * P : (jc + 1) * P],
                           Q_sb[hp : hp + DH, nch, ms], True, True)
                    nc.scalar.activation(
                        pt[:, g * 2 : (g + 1) * 2, :].rearrange("p a b -> p (a b)"),
                        sp[:], Exp, scale=float(SCALE),
                    )
                    if prev is not None:
                        for jc in (2 * g, 2 * g + 1):
                            mm(prev["pv"][0 : DH + 1, :],
                               prev["v"][:, jc, 0 : DH + 1],
                               prev["pt"][:, jc, :],
                               jc == 0, jc == S // P - 1)
                    if i < 16 and g in (3, 7):  # V-proj nh=1 PE filler
                        nxt = next(vfill, None)
                        if nxt is not None:
                            emit_v_group(nxt[0], nxt[1], 1)

            def stage_b(i):  # pvsb copy + reciprocal
                st = state[i]
                pvsb = smal.tile([DH + 1, MB], FP, tag="pvsb", name="pvsb")
                nc.vector.tensor_copy(pvsb[:], st["pv"][0 : DH + 1, :])
                r = smal.tile([1, MB], MD, tag="r", name="r")
                nc.vector.reciprocal(r[:], pvsb[DH : DH + 1, :])
                st["pvsb"], st["r"] = pvsb, r

            def stage_c(i):  # broadcast 1/r + normalize (+bv)
                st = state[i]
                rb = rbs.tile([DH, MB], FP, tag="rb", name="rb")
                mm(rb[:], ones_sb[:, 0:DH], st["r"][:], True, True)
                dst = AO_sb[st["hp"] : st["hp"] + DH, st["nch"], st["ms"]]
                nc.vector.tensor_mul(dst, st["pvsb"][0:DH, :], rb[:])
                nc.vector.tensor_scalar_add(
                    dst, dst, bv_sb[st["hp"] : st["hp"] + DH,
                                    st["nch"] : st["nch"] + 1])
                del state[i]

            N_IT = len(items)
            for i in range(N_IT):
                stage_a(i)
                if i >= 1:
                    stage_b(i - 1)
                if i >= 2:
                    stage_c(i - 2)
            last = N_IT - 1
            state[last]["pv"] = pvs.tile([P, MB], FP, tag="pv", name="pv")
            for jc in range(S // P):
                mm(state[last]["pv"][0 : DH + 1, :],
                   state[last]["v"][:, jc, 0 : DH + 1],
                   state[last]["pt"][:, jc, :],
                   jc == 0, jc == S // P - 1)
            stage_b(last)
            stage_c(last - 1)
            stage_c(last)

            # ---------------- phase D: output projection ----------------
            with (
                tc.tile_pool(name="yt", bufs=3) as ytp,
                tc.tile_pool(name="res", bufs=3) as resp,
            ):
                wo = wp.tile([P, NCH, D], MD, tag="w", name="wo")
                nc.sync.dma_start(wo[:], _r3(WoT))
                yT3 = _r3(yT)
                for ncx in range(NCH):
                    for mh in range(M // 512):
                        ms = slice(mh * 512, (mh + 1) * 512)
                        ps = pp.tile([P, 512], FP, tag="pp", name="ps")
                        for qc in range(NCH):
                            mm(ps[:], wo[:, qc, ncx * P : (ncx + 1) * P],
                               AO_sb[:, qc, ms], qc == 0, qc == NCH - 1)
                        yt = ytp.tile([P, 512], FP, tag="yt", name="yt")
                        nc.scalar.activation(yt[:], ps[:], Ident,
                                             bias=bo_sb[:, ncx : ncx + 1])
                        res = resp.tile([P, 512], FP, tag="res", name="res")
                        nc.sync.dma_start(res[:], xTr3[:, ncx, ms])
                        nc.vector.tensor_add(yt[:], yt[:], res[:])
                        nc.sync.dma_start(yT3[:, ncx, ms], yt[:])
    return nc


def _round_fp32r(a):
    """Round fp32 array to E8M11 (fp32r) with round-to-nearest-even."""
    b = np.ascontiguousarray(a, np.float32).view(np.uint32)
    lsb = (b >> np.uint32(12)) & np.uint32(1)
    r = (b + np.uint32(0x7FF) + lsb) & np.uint32(0xFFFFF000)
    return r.view(np.float32)


def _prep_inputs(x, Wq, bq, Wk, bk, Wv, bv, Wo, bo, round_r=True):
    f32 = np.float32
    rnd = _round_fp32r if round_r else (lambda a: np.ascontiguousarray(a, f32))
    WqT = rnd(np.asarray(Wq, f32).T)
    WkT = rnd(np.asarray(Wk, f32).T)
    WvT = rnd(np.asarray(Wv, f32).T)
    WoT = rnd(np.asarray(Wo, f32).T)
    bq, bk, bv, bo = (np.ascontiguousarray(np.asarray(a, f32)) for a in (bq, bk, bv, bo))
    in_maps = []
    for c in range(8):
        b, half = c // 2, c % 2
        xTb = np.ascontiguousarray(np.asarray(x[b], f32).T)  # [D, S]
        xTq = xTb[:, half * M : (half + 1) * M]
        in_maps.append({
            "xT": rnd(xTb),
            "xTq": rnd(xTq),
            "xTr": np.ascontiguousarray(xTq),
            "WqT": WqT, "WkT": WkT, "WvT": WvT, "WoT": WoT,
            "bq": bq, "bk": bk, "bv": bv, "bo": bo,
        })
    return in_maps


def run(inputs, trace=False, mm_dt=FR):
    key = str(mm_dt)
    if key not in _CACHED:
        _CACHED[key] = build_program(mm_dt)
    nc = _CACHED[key]
    in_maps = _prep_inputs(**inputs, round_r=(mm_dt == FR))
    exec_ns = None
    prof_info = None
    res = run_bass_kernel_spmd(nc, in_maps, list(range(8)), trace=trace)
    results = res.results
    if trace:
        exec_ns = res.exec_time_ns
        prof_info = res.profile_json
    out = np.empty((4, S, D), np.float32)
    for c in range(8):
        b, half = c // 2, c % 2
        out[b, half * M : (half + 1) * M, :] = results[c]["yT"].T
    return out, exec_ns, prof_info


def kernel(**inputs):
    out, _, _ = run(inputs, trace=False)
    return out


# revision 38
# speedup vs baseline: 1.2481x; 1.0750x over previous
"""Trainium2 Bass kernel for an attention block (dense transformer).

Reference computation (per batch b):
    q = x @ Wq.T + bq ; k = x @ Wk.T + bk ; v = x @ Wv.T + bv
    per head: attn = softmax(q k^T / sqrt(dh)) ; o = attn @ v
    out = concat(o) @ Wo.T + bo + x

Sharding: 8 cores = 4 batches x 2 query-halves (data parallel; K/V
projections duplicated within a pair, which avoids all collectives).

Device-side layouts are feature-major ("transposed"): the host passes
x[b].T and W.T so no on-device fp32 transposes are ever needed.
Matmuls run in float32r (fp32 rounded to E8M11, full-rate on the PE —
4x faster than plain fp32). Matmul operands coming from DRAM are
pre-rounded to fp32r on the host; operands produced on-device are
written with float32r output dtype by ACT/DVE so the BIR verifier's
"rounded producer" rule is satisfied.

Softmax: scoresT[j, m] = K.T-chunk.T @ Q.T, exp on ScalarE (no
max-subtraction needed: |scores| < ~3 for this distribution), and a
ones-column appended to V so P @ [V | 1] yields both P@V and the row
sums in one PSUM accumulation group. bv is folded in after
normalization (attn rows sum to 1).
"""

import os
import sys
from contextlib import ExitStack

import numpy as np

sys.path.insert(0, "/opt/trn_rl_repo")
os.environ.setdefault("MYCRO_LOCAL_CACHE", "1")

import concourse.bass as bass  # noqa: E402
import concourse.tile as tile  # noqa: E402
from concourse import mybir  # noqa: E402
from concourse.bass_utils import run_bass_kernel_spmd  # noqa: E402

# ---------------------------------------------------------------------------
# walrus codegen in this toolchain encodes at most ONE semaphore wait per
# instruction ("Too many sync wait commands").  Tile's scheduler freely emits
# several.  Split every multi-wait sync_info into standalone EventSemaphore
# wait instructions on the same engine, immediately before the instruction —
# semantically identical (engine sequencers execute them in program order).
# ---------------------------------------------------------------------------
import json as _json  # noqa: E402
import concourse.bass_utils as _bu  # noqa: E402
from concourse import bass2jax as _b2j  # noqa: E402

_orig_compile_bir_kernel = _bu.compile_bir_kernel


def _lower_multiwait_sync(bir_bytes):
    bir = _json.loads(bir_bytes)
    nsplit = 0
    for fn in bir.get("functions", []):
        for blk in fn.get("blocks", []):
            out = []
            for ins in blk["instructions"]:
                si = ins.get("sync_info")
                waits = (si or {}).get("on_wait") or []
                if len(waits) > 1:
                    for i, w in enumerate(waits[:-1]):
                        nsplit += 1
                        out.append({
                            "debug": ins.get("debug", 0),
                            "engine": ins["engine"],
                            "ins": [],
                            "outs": [],
                            "name": f"{ins['name']}w{i}",
                            "opcode": "EventSemaphore",
                            "sync_info": {"on_wait": [w], "on_update": []},
                        })
                    si["on_wait"] = [waits[-1]]
                out.append(ins)
            blk["instructions"] = out
    return _json.dumps(bir).encode(), nsplit


def _patched_compile_bir_kernel(bir_json, tmpdir, neff_name="file.neff"):
    bir_json, nsplit = _lower_multiwait_sync(bir_json)
    if nsplit:
        print(f"[kernel] split {nsplit} extra sync waits into standalone "
              f"EventSemaphore instructions", flush=True)
    return _orig_compile_bir_kernel(bir_json, tmpdir, neff_name)


_bu.compile_bir_kernel = _patched_compile_bir_kernel
_b2j.compile_bir_kernel = _patched_compile_bir_kernel

# ---------------------------------------------------------------------------
# NTFF profiling under axon: bass_utils wants antenv.axon_hooks (absent in
# this image) whose hook drives axon_{start,stop}_nrt_profile in
# libaxon_pjrt.so.  Recreate that shim here so trace=True works.
# ---------------------------------------------------------------------------
import contextlib as _contextlib  # noqa: E402
import ctypes as _ctypes  # noqa: E402
import types as _types  # noqa: E402

_AXON_SO = "/opt/axon/libaxon_pjrt.so"


def _make_ntff_hook():
    try:
        lib = _ctypes.CDLL(_AXON_SO)
    except OSError:
        return None
    if not hasattr(lib, "axon_start_nrt_profile"):
        return None
    lib.axon_start_nrt_profile.argtypes = [
        _ctypes.POINTER(_ctypes.c_int64), _ctypes.c_size_t]
    lib.axon_start_nrt_profile.restype = _ctypes.c_int64
    lib.axon_stop_nrt_profile.argtypes = [_ctypes.c_char_p]
    lib.axon_stop_nrt_profile.restype = _ctypes.c_int64

    @_contextlib.contextmanager
    def _hook(output_dir, device_ids):
        import jax

        jax.devices()  # force PJRT init so GLOBAL_CLIENT exists
        if device_ids:
            ids = (_ctypes.c_int64 * len(device_ids))(*device_ids)
            rc = lib.axon_start_nrt_profile(ids, len(device_ids))
        else:
            rc = lib.axon_start_nrt_profile(None, 0)
        if rc != 0:
            raise RuntimeError(f"axon_start_nrt_profile rc={rc}")
        try:
            yield
        finally:
            n = lib.axon_stop_nrt_profile(str(output_dir).encode())
            print(f"[kernel] ntff profile: {n} file(s) -> {output_dir}", flush=True)

    return _hook


if "antenv.axon_hooks" not in sys.modules:
    _m = _types.ModuleType("antenv.axon_hooks")
    _m.get_axon_ntff_profile_hook = _make_ntff_hook
    _m.set_axon_ntff_profile_hook = lambda h: None
    sys.modules["antenv.axon_hooks"] = _m

# the artifact upload wants a remote bucket; irrelevant here
_bu.upload_artifacts = lambda tmpdir: f"local:{tmpdir}"

P = 128
D = 1024  # model dim
S = 2048  # full sequence (keys per batch)
M = 1024  # queries per core (half a sequence)
H = 16  # heads
DH = 64  # head dim
NCH = D // P  # 8 feature chunks of 128
FP = mybir.dt.float32
FR = mybir.dt.float32r  # fast fp32 matmul mode (E8M11)
BF = mybir.dt.bfloat16  # attention-core matmul dtype (1 cyc/row + FWL)

Exp = mybir.ActivationFunctionType.Exp
Ident = mybir.ActivationFunctionType.Identity
SCALE = 1.0 / np.sqrt(DH)

_CACHED = {}


def _r3(ap):
    """[ (c p), f ] dram view -> [p, c, f]"""
    return ap.rearrange("(c p) f -> p c f", p=P)


def build_program(mm_dt=FR):
    nc = bass.Bass()
    MD = mm_dt
    xT = nc.dram_tensor("xT", [D, S], MD, kind="ExternalInput")
    xTq = nc.dram_tensor("xTq", [D, M], MD, kind="ExternalInput")
    xTr = nc.dram_tensor("xTr", [D, M], FP, kind="ExternalInput")  # residual
    WqT = nc.dram_tensor("WqT", [D, D], MD, kind="ExternalInput")
    WkT = nc.dram_tensor("WkT", [D, D], MD, kind="ExternalInput")
    WvT = nc.dram_tensor("WvT", [D, D], MD, kind="ExternalInput")
    WoT = nc.dram_tensor("WoT", [D, D], MD, kind="ExternalInput")
    bq = nc.dram_tensor("bq", [D], FP, kind="ExternalInput")
    bk = nc.dram_tensor("bk", [D], FP, kind="ExternalInput")
    bv = nc.dram_tensor("bv", [D], FP, kind="ExternalInput")
    bo = nc.dram_tensor("bo", [D], FP, kind="ExternalInput")
    yT = nc.dram_tensor("yT", [D, M], FP, kind="ExternalOutput")
    v_spill = nc.dram_tensor("v_spill", [H, S, DH], BF)

    def mm(ps, lhsT, rhs, start, stop):
        nc.tensor.matmul(ps, lhsT=lhsT, rhs=rhs, start=start, stop=stop)

    with tile.TileContext(nc) as tc, ExitStack() as ctx:
        ctx.enter_context(
            nc.allow_low_precision(reason="float32r (E8M11) matmul operands by design")
        )
        kq = ctx.enter_context(tc.tile_pool(name="kq", bufs=1))
        K_sb = kq.tile([P, NCH, S], BF, tag="K")  # K.T (bf16)  32KB/part
        Q_sb = kq.tile([P, NCH, M], BF, tag="Q")  # Q.T (bf16)  16KB/part
        bq_sb = kq.tile([P, NCH], FP, tag="bq")
        bk_sb = kq.tile([P, NCH], FP, tag="bk")
        bv_sb = kq.tile([P, NCH], FP, tag="bv")
        bo_sb = kq.tile([P, NCH], FP, tag="bo")
        for t, d in ((bq_sb, bq), (bk_sb, bk), (bv_sb, bv), (bo_sb, bo)):
            nc.sync.dma_start(t[:], d.rearrange("(c p) -> p c", p=P))

        xT3, xTq3, xTr3 = _r3(xT), _r3(xTq), _r3(xTr)

        # ------------------------------------------------------------------
        # Integrated schedule.  HAM throttles the PE array to 1.2 GHz unless
        # it stays nearly continuously busy, and attention alone has less PE
        # work than ScalarE exp work.  So the V-projection's second half is
        # interleaved INTO the attention stream as PE filler:
        #   A: K-proj, Q-proj, V-proj heads 0-7          (dense PE)
        #   B: attention heads 0-7  ||  V-proj heads 8-15 (2 groups/iter)
        #   C: attention heads 8-15
        #   D: output projection
        # Attention itself is a 2-deep software pipeline per (head, m-block):
        # previous iteration's PV matmuls are interleaved between score
        # groups; the reciprocal is consumed two iterations later.
        # ------------------------------------------------------------------
        MB = 512
        with (
            tc.tile_pool(name="w", bufs=1) as wp,
            tc.tile_pool(name="xs", bufs=2) as xp,
            tc.tile_pool(name="pp", bufs=2, space="PSUM") as pp,
            tc.tile_pool(name="vb", bufs=2) as vbp,
            tc.tile_pool(name="aop", bufs=1) as aop,
            tc.tile_pool(name="vp", bufs=2) as vp,
            tc.tile_pool(name="ptp", bufs=2) as ptp,
            tc.tile_pool(name="sps", bufs=2, space="PSUM") as sps,
            tc.tile_pool(name="pvs", bufs=1, space="PSUM") as pvs,
            tc.tile_pool(name="rbs", bufs=1, space="PSUM") as rbs,
            tc.tile_pool(name="smal", bufs=3) as smal,
            tc.tile_pool(name="yt", bufs=3) as ytp,
            tc.tile_pool(name="res", bufs=3) as resp,
        ):
            AO_sb = aop.tile([P, NCH, M], MD, tag="AO")  # attn out, feature-major
            ones_fp = aop.tile([P, S // P, 2], FP, tag="ones_fp")
            nc.any.memset(ones_fp[:], 1.0)
            ones1_fp = aop.tile([1, DH], FP, tag="ones1_fp")
            nc.any.memset(ones1_fp[:], 1.0)
            ones_sb = aop.tile([1, DH], MD, tag="ones")
            nc.vector.tensor_copy(ones_sb[:], ones1_fp[:])
            vd = v_spill.rearrange("h (jc p) d -> p jc h d", p=P)

            # --- phase A1: K.T = WkT.T @ xT (+bk), feature-major ---
            w = wp.tile([P, NCH, D], MD, tag="w", name="w")
            nc.sync.dma_start(w[:], _r3(WkT))
            for jb in range(S // 512):
                xs = xp.tile([P, NCH, 512], MD, tag="xs", name="xs")
                nc.sync.dma_start(xs[:], xT3[:, :, jb * 512 : (jb + 1) * 512])
                for ncx in range(NCH):
                    ps = pp.tile([P, 512], FP, tag="pp", name="ps")
                    for dc in range(NCH):
                        mm(ps[:], w[:, dc, ncx * P : (ncx + 1) * P], xs[:, dc, :],
                           dc == 0, dc == NCH - 1)
                    nc.vector.tensor_scalar_add(
                        K_sb[:, ncx, jb * 512 : (jb + 1) * 512], ps[:],
                        bk_sb[:, ncx : ncx + 1])
            # --- phase A2: Q.T (+bq), query half only ---
            w = wp.tile([P, NCH, D], MD, tag="w", name="w")
            nc.sync.dma_start(w[:], _r3(WqT))
            for jb in range(M // 512):
                xs = xp.tile([P, NCH, 512], MD, tag="xs", name="xs")
                nc.sync.dma_start(xs[:], xTq3[:, :, jb * 512 : (jb + 1) * 512])
                for ncx in range(NCH):
                    ps = pp.tile([P, 512], FP, tag="pp", name="ps")
                    for dc in range(NCH):
                        mm(ps[:], w[:, dc, ncx * P : (ncx + 1) * P], xs[:, dc, :],
                           dc == 0, dc == NCH - 1)
                    nc.vector.tensor_scalar_add(
                        Q_sb[:, ncx, jb * 512 : (jb + 1) * 512], ps[:],
                        bq_sb[:, ncx : ncx + 1])
            # --- V = xT-chunk.T @ WvT, natural [j, n]; spilled to DRAM.
            # One half (8 heads) at a time; nh=1 is emitted lazily as PE
            # filler inside the attention stream (phase B).
            wv = wp.tile([P, NCH, D], MD, tag="w", name="w")
            nc.sync.dma_start(wv[:], _r3(WvT))

            def v_group_iter(nh):
                for jb in range(S // 512):
                    xs = xp.tile([P, NCH, 512], MD, tag="xs", name="xs")
                    nc.sync.dma_start(xs[:], xT3[:, :, jb * 512 : (jb + 1) * 512])
                    for js in range(4):
                        yield (xs, jb * 4 + js)

            def emit_v_group(xs, jc, nh):
                ps = pp.tile([P, 512], FP, tag="pp", name="ps")
                js = jc % 4
                for dc in range(NCH):
                    mm(ps[:], xs[:, dc, js * P : (js + 1) * P],
                       wv[:, dc, nh * 512 : (nh + 1) * 512], dc == 0, dc == NCH - 1)
                vbt = vbp.tile([P, 512], BF, tag="vb", name="vbt")
                nc.vector.tensor_copy(vbt[:], ps[:])
                nc.sync.dma_start(vd[:, jc, nh * 8 : (nh + 1) * 8, :], vbt[:])

            for xs, jc in v_group_iter(0):  # phase A3: heads 0-7
                emit_v_group(xs, jc, 0)
            vfill = v_group_iter(1)  # phase B filler: heads 8-15

            # ---------------- attention pipeline ----------------
            items = [(h, mb) for h in range(H) for mb in range(M // MB)]
            state = {}

            def stage_a(i):
                h, mb = items[i]
                nch, hp = h // 2, (h % 2) * DH
                ms = slice(mb * MB, (mb + 1) * MB)
                if mb == 0:
                    v = vp.tile([P, S // P, DH + 2], BF, tag="v", name="v")
                    nc.vector.tensor_copy(v[:, :, DH : DH + 2], ones_fp[:])
                    nc.sync.dma_start(
                        v[:, :, 0:DH],
                        v_spill[h].rearrange("(jc p) d -> p jc d", p=P),
                    )
                else:
                    v = state[i - 1]["v"]
                pt = ptp.tile([P, S // P, MB], BF, tag="pt", name="pt")
                st = dict(h=h, nch=nch, hp=hp, ms=ms, v=v, pt=pt, pv=None)
                state[i] = st
                prev = state.get(i - 1)
                if prev is not None:
                    prev["pv"] = pvs.tile([P, MB], FP, tag="pv", name="pv")
                for g in range(8):  # groups of 2 j-chunks
                    sp = sps.tile([P, 1024], FP, tag="sp", name="sp")
                    for q in range(2):
                        jc = g * 2 + q
                        mm(sp[:, q * MB : (q + 1) * MB],
                           K_sb[hp : hp + DH, nch, jc * P : (jc + 1) * P],
                           Q_sb[hp : hp + DH, nch, ms], True, True)
                    nc.scalar.activation(
                        pt[:, g * 2 : (g + 1) * 2, :].rearrange("p a b -> p (a b)"),
                        sp[:], Exp, scale=float(SCALE),
                    )
                    if prev is not None:
                        for jc in (2 * g, 2 * g + 1):
                            mm(prev["pv"][0 : DH + 1, :],
                               prev["v"][:, jc, 0 : DH + 1],
                               prev["pt"][:, jc, :],
                               jc == 0, jc == S // P - 1)
                    if i < 16 and g in (3, 7):  # V-proj nh=1 PE filler
                        nxt = next(vfill, None)
                        if nxt is not None:
                            emit_v_group(nxt[0], nxt[1], 1)
                    if i >= 17 and g == 5 and ofill:  # O-proj half1 PE filler
                        ofill.pop(0)()

            def stage_b(i):  # pvsb copy + reciprocal
                st = state[i]
                pvsb = smal.tile([DH + 1, MB], FP, tag="pvsb", name="pvsb")
                nc.vector.tensor_copy(pvsb[:], st["pv"][0 : DH + 1, :])
                r = smal.tile([1, MB], MD, tag="r", name="r")
                nc.vector.reciprocal(r[:], pvsb[DH : DH + 1, :])
                st["pvsb"], st["r"] = pvsb, r

            def stage_c(i):  # broadcast 1/r + normalize (+bv)
                st = state[i]
                rb = rbs.tile([DH, MB], FP, tag="rb", name="rb")
                mm(rb[:], ones_sb[:, 0:DH], st["r"][:], True, True)
                dst = AO_sb[st["hp"] : st["hp"] + DH, st["nch"], st["ms"]]
                nc.vector.tensor_mul(dst, st["pvsb"][0:DH, :], rb[:])
                nc.vector.tensor_scalar_add(
                    dst, dst, bv_sb[st["hp"] : st["hp"] + DH,
                                    st["nch"] : st["nch"] + 1])
                del state[i]

            # ---- output projection, split for phase-C PE filler ----
            # half1 (qc 0-3, heads 0-7): bias + residual + first half of the
            # contraction, plain store to yT.  half2 (qc 4-7): accumulated
            # into yT via DMA read-modify-write at the tail.
            yT3 = _r3(yT)
            wo = wp.tile([P, NCH, D], MD, tag="w", name="wo")

            def oproj_half1(ncx, mh):
                ms = slice(mh * 512, (mh + 1) * 512)
                ps = pp.tile([P, 512], FP, tag="pp", name="ps")
                for qc in range(4):
                    mm(ps[:], wo[:, qc, ncx * P : (ncx + 1) * P],
                       AO_sb[:, qc, ms], qc == 0, qc == 3)
                yt = ytp.tile([P, 512], FP, tag="yt", name="yt")
                nc.vector.tensor_scalar_add(yt[:], ps[:], bo_sb[:, ncx : ncx + 1])
                res = resp.tile([P, 512], FP, tag="res", name="res")
                nc.sync.dma_start(res[:], xTr3[:, ncx, ms])
                nc.vector.tensor_add(yt[:], yt[:], res[:])
                nc.sync.dma_start(yT3[:, ncx, ms], yt[:])

            def oproj_half2(ncx, mh):
                ms = slice(mh * 512, (mh + 1) * 512)
                ps = pp.tile([P, 512], FP, tag="pp", name="ps")
                for qc in range(4, NCH):
                    mm(ps[:], wo[:, qc, ncx * P : (ncx + 1) * P],
                       AO_sb[:, qc, ms], qc == 4, qc == NCH - 1)
                yt = ytp.tile([P, 512], FP, tag="yt", name="yt")
                nc.vector.tensor_copy(yt[:], ps[:])
                nc.gpsimd.dma_start(yT3[:, ncx, ms], yt[:],
                                    accum_op=mybir.AluOpType.add)

            ofill = [
                (lambda ncx=ncx, mh=mh: oproj_half1(ncx, mh))
                for ncx in range(NCH) for mh in range(M // 512)
            ]
            wo_loaded = [False]

            N_IT = len(items)
            for i in range(N_IT):
                if i == 16 and not wo_loaded[0]:
                    nc.sync.dma_start(wo[:], _r3(WoT))
                    wo_loaded[0] = True
                stage_a(i)
                if i >= 1:
                    stage_b(i - 1)
                if i >= 2:
                    stage_c(i - 2)
            last = N_IT - 1
            state[last]["pv"] = pvs.tile([P, MB], FP, tag="pv", name="pv")
            for jc in range(S // P):
                mm(state[last]["pv"][0 : DH + 1, :],
                   state[last]["v"][:, jc, 0 : DH + 1],
                   state[last]["pt"][:, jc, :],
                   jc == 0, jc == S // P - 1)
            stage_b(last)
            stage_c(last - 1)
            stage_c(last)
            for f in ofill:  # any half1 units not used as filler
                f()
            for ncx in range(NCH):
                for mh in range(M // 512):
                    oproj_half2(ncx, mh)
    return nc


def _round_fp32r(a):
    """Round fp32 array to E8M11 (fp32r) with round-to-nearest-even."""
    b = np.ascontiguousarray(a, np.float32).view(np.uint32)
    lsb = (b >> np.uint32(12)) & np.uint32(1)
    r = (b + np.uint32(0x7FF) + lsb) & np.uint32(0xFFFFF000)
    return r.view(np.float32)


def _prep_inputs(x, Wq, bq, Wk, bk, Wv, bv, Wo, bo, round_r=True):
    f32 = np.float32
    rnd = _round_fp32r if round_r else (lambda a: np.ascontiguousarray(a, f32))
    WqT = rnd(np.asarray(Wq, f32).T)
    WkT = rnd(np.asarray(Wk, f32).T)
    WvT = rnd(np.asarray(Wv, f32).T)
    WoT = rnd(np.asarray(Wo, f32).T)
    bq, bk, bv, bo = (np.ascontiguousarray(np.asarray(a, f32)) for a in (bq, bk, bv, bo))
    in_maps = []
    for c in range(8):
        b, half = c // 2, c % 2
        xTb = np.ascontiguousarray(np.asarray(x[b], f32).T)  # [D, S]
        xTq = xTb[:, half * M : (half + 1) * M]
        in_maps.append({
            "xT": rnd(xTb),
            "xTq": rnd(xTq),
            "xTr": np.ascontiguousarray(xTq),
            "WqT": WqT, "WkT": WkT, "WvT": WvT, "WoT": WoT,
            "bq": bq, "bk": bk, "bv": bv, "bo": bo,
        })
    return in_maps


def run(inputs, trace=False, mm_dt=FR):
    key = str(mm_dt)
    if key not in _CACHED:
        _CACHED[key] = build_program(mm_dt)
    nc = _CACHED[key]
    in_maps = _prep_inputs(**inputs, round_r=(mm_dt == FR))
    exec_ns = None
    prof_info = None
    res = run_bass_kernel_spmd(nc, in_maps, list(range(8)), trace=trace)
    results = res.results
    if trace:
        exec_ns = res.exec_time_ns
        prof_info = res.profile_json
    out = np.empty((4, S, D), np.float32)
    for c in range(8):
        b, half = c // 2, c % 2
        out[b, half * M : (half + 1) * M, :] = results[c]["yT"].T
    return out, exec_ns, prof_info


def kernel(**inputs):
    out, _, _ = run(inputs, trace=False)
    return out


# revision 40
# speedup vs baseline: 1.3258x; 1.0622x over previous
"""Trainium2 Bass kernel for an attention block (dense transformer).

Reference computation (per batch b):
    q = x @ Wq.T + bq ; k = x @ Wk.T + bk ; v = x @ Wv.T + bv
    per head: attn = softmax(q k^T / sqrt(dh)) ; o = attn @ v
    out = concat(o) @ Wo.T + bo + x

Sharding: 8 cores = 4 batches x 2 query-halves (data parallel; K/V
projections duplicated within a pair, which avoids all collectives).

Device-side layouts are feature-major ("transposed"): the host passes
x[b].T and W.T so no on-device fp32 transposes are ever needed.
Matmuls run in float32r (fp32 rounded to E8M11, full-rate on the PE —
4x faster than plain fp32). Matmul operands coming from DRAM are
pre-rounded to fp32r on the host; operands produced on-device are
written with float32r output dtype by ACT/DVE so the BIR verifier's
"rounded producer" rule is satisfied.

Softmax: scoresT[j, m] = K.T-chunk.T @ Q.T, exp on ScalarE (no
max-subtraction needed: |scores| < ~3 for this distribution), and a
ones-column appended to V so P @ [V | 1] yields both P@V and the row
sums in one PSUM accumulation group. bv is folded in after
normalization (attn rows sum to 1).
"""

import os
import sys
from contextlib import ExitStack

import numpy as np

sys.path.insert(0, "/opt/trn_rl_repo")
os.environ.setdefault("MYCRO_LOCAL_CACHE", "1")

import concourse.bass as bass  # noqa: E402
import concourse.tile as tile  # noqa: E402
from concourse import mybir  # noqa: E402
from concourse.bass_utils import run_bass_kernel_spmd  # noqa: E402

# ---------------------------------------------------------------------------
# walrus codegen in this toolchain encodes at most ONE semaphore wait per
# instruction ("Too many sync wait commands").  Tile's scheduler freely emits
# several.  Split every multi-wait sync_info into standalone EventSemaphore
# wait instructions on the same engine, immediately before the instruction —
# semantically identical (engine sequencers execute them in program order).
# ---------------------------------------------------------------------------
import json as _json  # noqa: E402
import concourse.bass_utils as _bu  # noqa: E402
from concourse import bass2jax as _b2j  # noqa: E402

_orig_compile_bir_kernel = _bu.compile_bir_kernel


def _lower_multiwait_sync(bir_bytes):
    bir = _json.loads(bir_bytes)
    nsplit = 0
    for fn in bir.get("functions", []):
        for blk in fn.get("blocks", []):
            out = []
            for ins in blk["instructions"]:
                si = ins.get("sync_info")
                waits = (si or {}).get("on_wait") or []
                if len(waits) > 1:
                    for i, w in enumerate(waits[:-1]):
                        nsplit += 1
                        out.append({
                            "debug": ins.get("debug", 0),
                            "engine": ins["engine"],
                            "ins": [],
                            "outs": [],
                            "name": f"{ins['name']}w{i}",
                            "opcode": "EventSemaphore",
                            "sync_info": {"on_wait": [w], "on_update": []},
                        })
                    si["on_wait"] = [waits[-1]]
                out.append(ins)
            blk["instructions"] = out
    return _json.dumps(bir).encode(), nsplit


def _patched_compile_bir_kernel(bir_json, tmpdir, neff_name="file.neff"):
    bir_json, nsplit = _lower_multiwait_sync(bir_json)
    if nsplit:
        print(f"[kernel] split {nsplit} extra sync waits into standalone "
              f"EventSemaphore instructions", flush=True)
    return _orig_compile_bir_kernel(bir_json, tmpdir, neff_name)


_bu.compile_bir_kernel = _patched_compile_bir_kernel
_b2j.compile_bir_kernel = _patched_compile_bir_kernel

# ---------------------------------------------------------------------------
# NTFF profiling under axon: bass_utils wants antenv.axon_hooks (absent in
# this image) whose hook drives axon_{start,stop}_nrt_profile in
# libaxon_pjrt.so.  Recreate that shim here so trace=True works.
# ---------------------------------------------------------------------------
import contextlib as _contextlib  # noqa: E402
import ctypes as _ctypes  # noqa: E402
import types as _types  # noqa: E402

_AXON_SO = "/opt/axon/libaxon_pjrt.so"


def _make_ntff_hook():
    try:
        lib = _ctypes.CDLL(_AXON_SO)
    except OSError:
        return None
    if not hasattr(lib, "axon_start_nrt_profile"):
        return None
    lib.axon_start_nrt_profile.argtypes = [
        _ctypes.POINTER(_ctypes.c_int64), _ctypes.c_size_t]
    lib.axon_start_nrt_profile.restype = _ctypes.c_int64
    lib.axon_stop_nrt_profile.argtypes = [_ctypes.c_char_p]
    lib.axon_stop_nrt_profile.restype = _ctypes.c_int64

    @_contextlib.contextmanager
    def _hook(output_dir, device_ids):
        import jax

        jax.devices()  # force PJRT init so GLOBAL_CLIENT exists
        if device_ids:
            ids = (_ctypes.c_int64 * len(device_ids))(*device_ids)
            rc = lib.axon_start_nrt_profile(ids, len(device_ids))
        else:
            rc = lib.axon_start_nrt_profile(None, 0)
        if rc != 0:
            raise RuntimeError(f"axon_start_nrt_profile rc={rc}")
        try:
            yield
        finally:
            n = lib.axon_stop_nrt_profile(str(output_dir).encode())
            print(f"[kernel] ntff profile: {n} file(s) -> {output_dir}", flush=True)

    return _hook


if "antenv.axon_hooks" not in sys.modules:
    _m = _types.ModuleType("antenv.axon_hooks")
    _m.get_axon_ntff_profile_hook = _make_ntff_hook
    _m.set_axon_ntff_profile_hook = lambda h: None
    sys.modules["antenv.axon_hooks"] = _m

# the artifact upload wants a remote bucket; irrelevant here
_bu.upload_artifacts = lambda tmpdir: f"local:{tmpdir}"

P = 128
D = 1024  # model dim
S = 2048  # full sequence (keys per batch)
M = 1024  # queries per core (half a sequence)
H = 16  # heads
DH = 64  # head dim
NCH = D // P  # 8 feature chunks of 128
FP = mybir.dt.float32
FR = mybir.dt.float32r  # fast fp32 matmul mode (E8M11)
BF = mybir.dt.bfloat16  # attention-core matmul dtype (1 cyc/row + FWL)

Exp = mybir.ActivationFunctionType.Exp
Ident = mybir.ActivationFunctionType.Identity
SCALE = 1.0 / np.sqrt(DH)

_CACHED = {}


def _r3(ap):
    """[ (c p), f ] dram view -> [p, c, f]"""
    return ap.rearrange("(c p) f -> p c f", p=P)


def build_program(mm_dt=FR):
    nc = bass.Bass()
    MD = mm_dt
    xT = nc.dram_tensor("xT", [D, S], MD, kind="ExternalInput")
    xTq = nc.dram_tensor("xTq", [D, M], MD, kind="ExternalInput")
    xTr = nc.dram_tensor("xTr", [D, M], FP, kind="ExternalInput")  # residual
    WqT = nc.dram_tensor("WqT", [D, D], MD, kind="ExternalInput")
    WkT = nc.dram_tensor("WkT", [D, D], MD, kind="ExternalInput")
    WvT = nc.dram_tensor("WvT", [D, D], MD, kind="ExternalInput")
    WoT = nc.dram_tensor("WoT", [D, D], MD, kind="ExternalInput")
    bq = nc.dram_tensor("bq", [D], FP, kind="ExternalInput")
    bk = nc.dram_tensor("bk", [D], FP, kind="ExternalInput")
    bv = nc.dram_tensor("bv", [D], FP, kind="ExternalInput")
    bo = nc.dram_tensor("bo", [D], FP, kind="ExternalInput")
    yT = nc.dram_tensor("yT", [D, M], FP, kind="ExternalOutput")
    v_spill = nc.dram_tensor("v_spill", [H, S, DH], BF)

    def mm(ps, lhsT, rhs, start, stop):
        nc.tensor.matmul(ps, lhsT=lhsT, rhs=rhs, start=start, stop=stop)

    with tile.TileContext(nc) as tc, ExitStack() as ctx:
        ctx.enter_context(
            nc.allow_low_precision(reason="float32r (E8M11) matmul operands by design")
        )
        kq = ctx.enter_context(tc.tile_pool(name="kq", bufs=1))
        K_sb = kq.tile([P, NCH, S], BF, tag="K")  # K.T (bf16)  32KB/part
        Q_sb = kq.tile([P, NCH, M], BF, tag="Q")  # Q.T (bf16)  16KB/part
        bq_sb = kq.tile([P, NCH], FP, tag="bq")
        bk_sb = kq.tile([P, NCH], FP, tag="bk")
        bv_sb = kq.tile([P, NCH], FP, tag="bv")
        bo_sb = kq.tile([P, NCH], FP, tag="bo")
        for t, d in ((bq_sb, bq), (bk_sb, bk), (bv_sb, bv), (bo_sb, bo)):
            nc.sync.dma_start(t[:], d.rearrange("(c p) -> p c", p=P))

        xT3, xTq3, xTr3 = _r3(xT), _r3(xTq), _r3(xTr)

        # ------------------------------------------------------------------
        # Integrated schedule.  HAM throttles the PE array to 1.2 GHz unless
        # it stays nearly continuously busy, and attention alone has less PE
        # work than ScalarE exp work.  So the V-projection's second half is
        # interleaved INTO the attention stream as PE filler:
        #   A: K-proj, Q-proj, V-proj heads 0-7          (dense PE)
        #   B: attention heads 0-7  ||  V-proj heads 8-15 (2 groups/iter)
        #   C: attention heads 8-15
        #   D: output projection
        # Attention itself is a 2-deep software pipeline per (head, m-block):
        # previous iteration's PV matmuls are interleaved between score
        # groups; the reciprocal is consumed two iterations later.
        # ------------------------------------------------------------------
        MB = 512
        with (
            tc.tile_pool(name="w", bufs=1) as wp,
            tc.tile_pool(name="xs", bufs=2) as xp,
            tc.tile_pool(name="pp", bufs=2, space="PSUM") as pp,
            tc.tile_pool(name="vb", bufs=2) as vbp,
            tc.tile_pool(name="aop", bufs=1) as aop,
            tc.tile_pool(name="vp", bufs=2) as vp,
            tc.tile_pool(name="ptp", bufs=2) as ptp,
            tc.tile_pool(name="sps", bufs=2, space="PSUM") as sps,
            tc.tile_pool(name="pvs", bufs=1, space="PSUM") as pvs,
            tc.tile_pool(name="rbs", bufs=1, space="PSUM") as rbs,
            tc.tile_pool(name="smal", bufs=3) as smal,
            tc.tile_pool(name="yt", bufs=3) as ytp,
            tc.tile_pool(name="res", bufs=3) as resp,
        ):
            AO_sb = aop.tile([P, NCH, M], MD, tag="AO")  # attn out, feature-major
            ones_fp = aop.tile([P, S // P, 2], FP, tag="ones_fp")
            nc.any.memset(ones_fp[:], 1.0)
            ones1_fp = aop.tile([1, DH], FP, tag="ones1_fp")
            nc.any.memset(ones1_fp[:], 1.0)
            ones_sb = aop.tile([1, DH], MD, tag="ones")
            nc.vector.tensor_copy(ones_sb[:], ones1_fp[:])
            vd = v_spill.rearrange("h (jc p) d -> p jc h d", p=P)

            # --- phase A1: K.T = WkT.T @ xT (+bk), feature-major ---
            w = wp.tile([P, NCH, D], MD, tag="w", name="w")
            nc.sync.dma_start(w[:], _r3(WkT))
            for jb in range(S // 512):
                xs = xp.tile([P, NCH, 512], MD, tag="xs", name="xs")
                nc.sync.dma_start(xs[:], xT3[:, :, jb * 512 : (jb + 1) * 512])
                for ncx in range(NCH):
                    ps = pp.tile([P, 512], FP, tag="pp", name="ps")
                    for dc in range(NCH):
                        mm(ps[:], w[:, dc, ncx * P : (ncx + 1) * P], xs[:, dc, :],
                           dc == 0, dc == NCH - 1)
                    nc.vector.tensor_scalar_add(
                        K_sb[:, ncx, jb * 512 : (jb + 1) * 512], ps[:],
                        bk_sb[:, ncx : ncx + 1])
            # --- phase A2: Q.T (+bq), query half only ---
            w = wp.tile([P, NCH, D], MD, tag="w", name="w")
            nc.sync.dma_start(w[:], _r3(WqT))
            for jb in range(M // 512):
                xs = xp.tile([P, NCH, 512], MD, tag="xs", name="xs")
                nc.sync.dma_start(xs[:], xTq3[:, :, jb * 512 : (jb + 1) * 512])
                for ncx in range(NCH):
                    ps = pp.tile([P, 512], FP, tag="pp", name="ps")
                    for dc in range(NCH):
                        mm(ps[:], w[:, dc, ncx * P : (ncx + 1) * P], xs[:, dc, :],
                           dc == 0, dc == NCH - 1)
                    nc.vector.tensor_scalar_add(
                        Q_sb[:, ncx, jb * 512 : (jb + 1) * 512], ps[:],
                        bq_sb[:, ncx : ncx + 1])
            # --- V = xT-chunk.T @ WvT, natural [j, n]; spilled to DRAM.
            # One QUARTER (4 heads, N=256) at a time: quarter 0 up front,
            # quarters 1-3 emitted lazily as PE filler inside the attention
            # stream, each completing just before the heads that need it.
            wv = wp.tile([P, NCH, D], MD, tag="w", name="w")
            nc.sync.dma_start(wv[:], _r3(WvT))

            def v_group_iter(q):
                for jb in range(S // 512):
                    xs = xp.tile([P, NCH, 512], MD, tag="xs", name="xs")
                    nc.sync.dma_start(xs[:], xT3[:, :, jb * 512 : (jb + 1) * 512])
                    for js in range(4):
                        yield (xs, jb * 4 + js)

            def emit_v_group(xs, jc, q):
                ps = pp.tile([P, 256], FP, tag="pp", name="ps")
                js = jc % 4
                for dc in range(NCH):
                    mm(ps[:], xs[:, dc, js * P : (js + 1) * P],
                       wv[:, dc, q * 256 : (q + 1) * 256], dc == 0, dc == NCH - 1)
                vbt = vbp.tile([P, 256], BF, tag="vb", name="vbt")
                nc.vector.tensor_copy(vbt[:], ps[:])
                nc.sync.dma_start(vd[:, jc, q * 4 : (q + 1) * 4, :], vbt[:])

            for xs, jc in v_group_iter(0):  # phase A3: heads 0-3
                emit_v_group(xs, jc, 0)

            # ---------------- attention pipeline ----------------
            items = [(h, mb) for h in range(H) for mb in range(M // MB)]
            state = {}

            def stage_a(i):
                h, mb = items[i]
                nch, hp = h // 2, (h % 2) * DH
                ms = slice(mb * MB, (mb + 1) * MB)
                if mb == 0:
                    v = vp.tile([P, S // P, DH + 2], BF, tag="v", name="v")
                    nc.vector.tensor_copy(v[:, :, DH : DH + 2], ones_fp[:])
                    nc.sync.dma_start(
                        v[:, :, 0:DH],
                        v_spill[h].rearrange("(jc p) d -> p jc d", p=P),
                    )
                else:
                    v = state[i - 1]["v"]
                pt = ptp.tile([P, S // P, MB], BF, tag="pt", name="pt")
                st = dict(h=h, nch=nch, hp=hp, ms=ms, v=v, pt=pt, pv=None)
                state[i] = st
                prev = state.get(i - 1)
                if prev is not None:
                    prev["pv"] = pvs.tile([P, MB], FP, tag="pv", name="pv")
                for g in range(8):  # groups of 2 j-chunks
                    sp = sps.tile([P, 1024], FP, tag="sp", name="sp")
                    for q in range(2):
                        jc = g * 2 + q
                        mm(sp[:, q * MB : (q + 1) * MB],
                           K_sb[hp : hp + DH, nch, jc * P : (jc + 1) * P],
                           Q_sb[hp : hp + DH, nch, ms], True, True)
                    nc.scalar.activation(
                        pt[:, g * 2 : (g + 1) * 2, :].rearrange("p a b -> p (a b)"),
                        sp[:], Exp, scale=float(SCALE),
                    )
                    if prev is not None:
                        for jc in (2 * g, 2 * g + 1):
                            mm(prev["pv"][0 : DH + 1, :],
                               prev["v"][:, jc, 0 : DH + 1],
                               prev["pt"][:, jc, :],
                               jc == 0, jc == S // P - 1)
                    fl = filler_sched.get(i)
                    if fl and g in (2, 3, 5, 7):
                        fl.pop(0)()
                        if not fl:
                            del filler_sched[i]

            def stage_b(i):  # pvsb copy + reciprocal
                st = state[i]
                pvsb = smal.tile([DH + 1, MB], FP, tag="pvsb", name="pvsb")
                nc.vector.tensor_copy(pvsb[:], st["pv"][0 : DH + 1, :])
                r = smal.tile([1, MB], MD, tag="r", name="r")
                nc.vector.reciprocal(r[:], pvsb[DH : DH + 1, :])
                st["pvsb"], st["r"] = pvsb, r

            def stage_c(i):  # broadcast 1/r + normalize (+bv)
                st = state[i]
                rb = rbs.tile([DH, MB], FP, tag="rb", name="rb")
                mm(rb[:], ones_sb[:, 0:DH], st["r"][:], True, True)
                dst = AO_sb[st["hp"] : st["hp"] + DH, st["nch"], st["ms"]]
                nc.vector.tensor_mul(dst, st["pvsb"][0:DH, :], rb[:])
                nc.vector.tensor_scalar_add(
                    dst, dst, bv_sb[st["hp"] : st["hp"] + DH,
                                    st["nch"] : st["nch"] + 1])
                del state[i]

            # ---- output projection, split for phase-C PE filler ----
            # main part (qc 0-5, heads 0-11): bias + residual + most of the
            # contraction, plain store to yT, interleaved into the last
            # attention iterations.  tail (qc 6-7): DMA read-modify-write.
            yT3 = _r3(yT)
            wo = wp.tile([P, NCH, D], MD, tag="w", name="wo")

            def oproj_main(ncx, mh):
                ms = slice(mh * 512, (mh + 1) * 512)
                ps = pp.tile([P, 512], FP, tag="pp", name="ps")
                for qc in range(6):
                    mm(ps[:], wo[:, qc, ncx * P : (ncx + 1) * P],
                       AO_sb[:, qc, ms], qc == 0, qc == 5)
                yt = ytp.tile([P, 512], FP, tag="yt", name="yt")
                nc.vector.tensor_scalar_add(yt[:], ps[:], bo_sb[:, ncx : ncx + 1])
                res = resp.tile([P, 512], FP, tag="res", name="res")
                nc.sync.dma_start(res[:], xTr3[:, ncx, ms])
                nc.vector.tensor_add(yt[:], yt[:], res[:])
                nc.sync.dma_start(yT3[:, ncx, ms], yt[:])

            def oproj_tail(ncx, mh):
                ms = slice(mh * 512, (mh + 1) * 512)
                ps = pp.tile([P, 512], FP, tag="pp", name="ps")
                for qc in range(6, NCH):
                    mm(ps[:], wo[:, qc, ncx * P : (ncx + 1) * P],
                       AO_sb[:, qc, ms], qc == 6, qc == NCH - 1)
                yt = ytp.tile([P, 512], FP, tag="yt", name="yt")
                nc.vector.tensor_copy(yt[:], ps[:])
                nc.gpsimd.dma_start(yT3[:, ncx, ms], yt[:],
                                    accum_op=mybir.AluOpType.add)

            # PE-filler schedule: quarter q of V-proj spread over the 8
            # iterations before the heads that consume it; O-proj main units
            # once qc<=5 inputs are normalized (emission-order safe: stage_c
            # for item 23 is emitted during loop step 25).
            filler_sched = {}
            for q in (1, 2, 3):
                it = v_group_iter(q)
                for step in range(8 * (q - 1), 8 * q):
                    filler_sched.setdefault(step, [])
                    for _ in range(2):
                        nxt = next(it, None)
                        if nxt is not None:
                            filler_sched[step].append(
                                lambda xs=nxt[0], jc=nxt[1], q=q: emit_v_group(xs, jc, q))
            ounits = [(ncx, mh) for ncx in range(NCH) for mh in range(M // 512)]
            oi = 0
            for step in range(26, 32):
                filler_sched.setdefault(step, [])
                for _ in range(3):
                    if oi < len(ounits):
                        ncx, mh = ounits[oi]
                        filler_sched[step].append(
                            lambda ncx=ncx, mh=mh: oproj_main(ncx, mh))
                        oi += 1
            oremain = ounits[oi:]
            wo_loaded = [False]

            N_IT = len(items)
            for i in range(N_IT):
                if i == 20 and not wo_loaded[0]:
                    nc.sync.dma_start(wo[:], _r3(WoT))
                    wo_loaded[0] = True
                stage_a(i)
                if i >= 1:
                    stage_b(i - 1)
                if i >= 2:
                    stage_c(i - 2)
            last = N_IT - 1
            state[last]["pv"] = pvs.tile([P, MB], FP, tag="pv", name="pv")
            for jc in range(S // P):
                mm(state[last]["pv"][0 : DH + 1, :],
                   state[last]["v"][:, jc, 0 : DH + 1],
                   state[last]["pt"][:, jc, :],
                   jc == 0, jc == S // P - 1)
            stage_b(last)
            stage_c(last - 1)
            stage_c(last)
            for ncx, mh in oremain:  # any main units not used as filler
                oproj_main(ncx, mh)
            for ncx in range(NCH):
                for mh in range(M // 512):
                    oproj_tail(ncx, mh)
    return nc


def _round_fp32r(a):
    """Round fp32 array to E8M11 (fp32r) with round-to-nearest-even."""
    b = np.ascontiguousarray(a, np.float32).view(np.uint32)
    lsb = (b >> np.uint32(12)) & np.uint32(1)
    r = (b + np.uint32(0x7FF) + lsb) & np.uint32(0xFFFFF000)
    return r.view(np.float32)


def _prep_inputs(x, Wq, bq, Wk, bk, Wv, bv, Wo, bo, round_r=True):
    f32 = np.float32
    rnd = _round_fp32r if round_r else (lambda a: np.ascontiguousarray(a, f32))
    WqT = rnd(np.asarray(Wq, f32).T)
    WkT = rnd(np.asarray(Wk, f32).T)
    WvT = rnd(np.asarray(Wv, f32).T)
    WoT = rnd(np.asarray(Wo, f32).T)
    bq, bk, bv, bo = (np.ascontiguousarray(np.asarray(a, f32)) for a in (bq, bk, bv, bo))
    in_maps = []
    for c in range(8):
        b, half = c // 2, c % 2
        xTb = np.ascontiguousarray(np.asarray(x[b], f32).T)  # [D, S]
        xTq = xTb[:, half * M : (half + 1) * M]
        in_maps.append({
            "xT": rnd(xTb),
            "xTq": rnd(xTq),
            "xTr": np.ascontiguousarray(xTq),
            "WqT": WqT, "WkT": WkT, "WvT": WvT, "WoT": WoT,
            "bq": bq, "bk": bk, "bv": bv, "bo": bo,
        })
    return in_maps


def run(inputs, trace=False, mm_dt=FR):
    key = str(mm_dt)
    if key not in _CACHED:
        _CACHED[key] = build_program(mm_dt)
    nc = _CACHED[key]
    in_maps = _prep_inputs(**inputs, round_r=(mm_dt == FR))
    exec_ns = None
    prof_info = None
    res = run_bass_kernel_spmd(nc, in_maps, list(range(8)), trace=trace)
    results = res.results
    if trace:
        exec_ns = res.exec_time_ns
        prof_info = res.profile_json
    out = np.empty((4, S, D), np.float32)
    for c in range(8):
        b, half = c // 2, c % 2
        out[b, half * M : (half + 1) * M, :] = results[c]["yT"].T
    return out, exec_ns, prof_info


def kernel(**inputs):
    out, _, _ = run(inputs, trace=False)
    return out
